# revision 24
# baseline (speedup 1.0000x reference)
"""Trainium2 Bass kernel for nn_Graph_Enhance_model (GNN message passing).

Self-contained: hardcodes shapes B=4,F=32,H=8,O=16,D=2048, 8 cores.
Phase A (edge waves): data-parallel over the 128 (b,f) frames, 16/core.
Phases B/C (GRUs): tensor-parallel over the hidden dim (256 hidden
units per core, 768 of the 6144 gate rows), stitched with AllGathers.
"""

import os
import sys

for _p in ("/opt/trn_rl_repo", "/opt/pypackages"):
    if _p not in sys.path and os.path.isdir(_p):
        sys.path.append(_p)

import numpy as np
import ml_dtypes

import concourse.bass as bass
import concourse.bacc as bacc
import concourse.tile as tile
import concourse.mybir as mybir
from concourse import bass_utils
from concourse.masks import make_identity

BF16 = mybir.dt.bfloat16
F8 = mybir.dt.float8e4
F32 = mybir.dt.float32
DR = mybir.MatmulPerfMode.DoubleRow
AF = mybir.ActivationFunctionType
ALU = mybir.AluOpType
AX = mybir.AxisListType

NB = ml_dtypes.bfloat16
NF8 = ml_dtypes.float8_e4m3
KC2 = 8  # D-chunk pairs for fp8 DoubleRow

B, F, H, O, D = 4, 32, 8, 16, 2048
NFRAMES = B * F          # 128
NCORES = 8
FPC = NFRAMES // NCORES  # 16 frames per core (phase A)
ROWS = H * O             # 128 edge rows per frame
KC = D // 128            # 16 K-chunks
NQ = FPC // 4            # 4 quads of 4 frames
HS = D // NCORES         # 256 hidden units per core (phases B/C)
NR = NFRAMES * H         # 1024 human rows globally
RC = NR // 128           # 8 row-chunks of 128

_CACHE = {}


def _combine_e(nc, step, mt, q, pe, wb_e, bet_sb, um8, msum_b, pool):
    """UM = (msg_e_psum + be) * w ; step 1 also reduces over o into msum.

    Scales: step 0 pe = 64*x, bet = 64*be, wb_e = w/4 -> out = 16*UM (fp8).
            step 1 pe = 1024*x, bet = 1024*be, wb_e = w/1024 -> out = UM.
    """
    if step == 0:
        nc.vector.scalar_tensor_tensor(out=um8[:, mt // 2, mt % 2, :], in0=pe,
                                       scalar=bet_sb[:, mt:mt + 1], in1=wb_e,
                                       op0=ALU.add, op1=ALU.mult)
    else:
        tmp = pool.tile([128, 512], F32, tag="um2")
        nc.vector.scalar_tensor_tensor(out=tmp, in0=pe,
                                       scalar=bet_sb[:, mt:mt + 1], in1=wb_e,
                                       op0=ALU.add, op1=ALU.mult)
        with nc.allow_low_precision(reason="msum store bf16; o-reduce of 16 terms"):
            nc.vector.reduce_sum(msum_b[:, mt, q * 32:(q + 1) * 32],
                                 tmp.rearrange("p (f h o) -> p f h o", f=4, h=8),
                                 axis=AX.X)


def _build_nc():
    nc = bacc.Bacc("TRN2", target_bir_lowering=False, debug=False, num_devices=NCORES)

    dt_in = {}

    def din(name, shape, dt):
        dt_in[name] = nc.dram_tensor(name, shape, dt, kind="ExternalInput")
        return dt_in[name]

    # ---- phase A inputs (per-core frames) ----
    e0t = din("e0t", [NQ, D, 512], F8)
    ot = din("ot", [D, FPC * O], F8)
    wcat = din("wcat", [D, D], F8)        # 64*[We | Wl1]^T
    bl1td = din("bl1t", [128, 8], F32)    # 64*bl1
    betd64 = din("bet64", [128, 8], F32)  # 64*be
    betd1k = din("bet1k", [128, 8], F32)  # 1024*be
    wnt = din("wnt", [D, D // 2], F8)     # 64*Wn^T
    wnb = din("wnb", [1, D // 2], BF16)   # 64*bn
    wl2 = din("wl2", [128, 8], BF16)
    # ---- phase B inputs (all frames; per-core hidden slice) ----
    htg = din("htg", [D, NR], BF16)          # H_nodes^T, all 128 frames
    pmatd = din("pmat", [NR, NFRAMES], BF16)  # mean-over-H matrix /8
    hrms = din("hrms", [NR, HS], F32)        # H_nodes rows, hidden slice
    wBi_d = din("wBi", [D, 3 * HS], BF16)    # (gh_wih/16)[rzn slice]^T
    wBh_d = din("wBh", [D, 3 * HS], BF16)    # gh_whh[rzn slice]^T
    bB_rz_d = din("bB_rz", [1, 2 * HS], BF16)
    bB_in_d = din("bB_in", [1, HS], BF16)
    bB_hn_d = din("bB_hn", [1, HS], BF16)
    # ---- phase C inputs ----
    scsf_d = din("scsf", [D, 2 * NFRAMES], BF16)   # [S_C4^T | S_f^T]
    sc4s_d = din("sc4s", [NFRAMES, HS], F32)
    sfs_d = din("sfs", [NFRAMES, HS], F32)
    wCi_d = din("wCi", [D, 3 * HS], BF16)
    wCh_d = din("wCh", [D, 3 * HS], BF16)
    bC_rz_d = din("bC_rz", [1, 2 * HS], BF16)      # (bih+bhh)[rz]
    bC_in_d = din("bC_in", [1, HS], BF16)
    bC_hn_d = din("bC_hn", [1, HS], BF16)

    outp = nc.dram_tensor("outp", [NFRAMES, HS], F32, kind="ExternalOutput")

    from contextlib import ExitStack

    RG = [list(range(NCORES))]

    with tile.TileContext(nc) as tc, ExitStack() as ctx:
        glob = ctx.enter_context(tc.tile_pool(name="glob", bufs=1))
        dram = ctx.enter_context(tc.tile_pool(name="dram", bufs=1, space="DRAM"))

        ag1_ins = [dram.tile([D, 32], BF16, name=f"ag1i{q}") for q in range(NQ)]
        ag1_outs = [dram.tile([NCORES * D, 32], BF16, addr_space="Shared",
                              name=f"ag1o{q}") for q in range(NQ)]
        ag2_in = dram.tile([2 * 128, NFRAMES], BF16)
        ag2_out = dram.tile([D, NFRAMES], BF16, addr_space="Shared")
        ag3_in = dram.tile([2 * 128, NFRAMES], BF16)
        ag3_out = dram.tile([D, NFRAMES], BF16, addr_space="Shared")

        ones_b = glob.tile([1, 512], BF16)
        nc.vector.memset(ones_b, 1.0)
        cst_q = glob.tile([1, 128], BF16)     # 1/4: step-0 e-half w scale
        nc.vector.memset(cst_q, 0.25)
        cst_16 = glob.tile([1, 128], BF16)    # 16: step-0 n-half w scale
        nc.vector.memset(cst_16, 16.0)
        cst_1k = glob.tile([1, 128], BF16)    # 1/1024: step-1 e-half w scale
        nc.vector.memset(cst_1k, 1.0 / 1024.0)
        ident = glob.tile([128, 128], BF16)
        make_identity(nc, ident)

        wl2_sb = glob.tile([128, 8], BF16)
        nc.sync.dma_start(out=wl2_sb, in_=wl2.ap())
        bl1t_sb = glob.tile([128, 8], F32)
        nc.sync.dma_start(out=bl1t_sb, in_=bl1td.ap())
        bet64_sb = glob.tile([128, 8], F32)
        nc.sync.dma_start(out=bet64_sb, in_=betd64.ap())
        bet1k_sb = glob.tile([128, 8], F32)
        nc.sync.dma_start(out=bet1k_sb, in_=betd1k.ap())

        msgn_sb = glob.tile([128, 8, FPC * O], BF16)    # [1024, 256] transposed msg_n
        msum_b = glob.tile([128, KC, FPC * H], BF16)    # M_sum2^T (raw sum over o)

        with (
            tc.tile_pool(name="pwcat", bufs=1) as pwcat,
            tc.tile_pool(name="pa", bufs=2) as pa,
            tc.tile_pool(name="pa1", bufs=1) as pa1,
        ):
            # ---------------- Phase 0: msg_n^T = Wn @ O^T + bn (fp8, x64) ----------------
            wcat_sb = pwcat.tile([128, KC2, 2, D], F8)
            with (
                tc.tile_pool(name="p0", bufs=1) as p0,
                tc.tile_pool(name="p0ps", bufs=4, space="PSUM") as p0ps,
            ):
                wnb_sb = p0.tile([1, D // 2], BF16)
                nc.sync.dma_start(out=wnb_sb, in_=wnb.ap())
                ot_sb = p0.tile([128, KC2, 2, FPC * O], F8)
                nc.sync.dma_start(out=ot_sb, in_=ot.ap()
                                  .rearrange("(kc2 two p) n -> p kc2 two n", p=128, two=2))
                wn_sb = p0.tile([128, KC2, 2, D // 2], F8)
                nc.sync.dma_start(out=wn_sb, in_=wnt.ap()
                                  .rearrange("(kc2 two p) m -> p kc2 two m", p=128, two=2))
                # wcat load issued after phase-0 inputs so PE can start sooner
                nc.sync.dma_start(out=wcat_sb,
                                  in_=wcat.ap().rearrange("(kc2 two p) m -> p kc2 two m",
                                                          p=128, two=2))
                for mt in range(8):
                    pm = p0ps.tile([128, FPC * O], F32, tag="pm")
                    for kc2 in range(KC2):
                        nc.tensor.matmul(pm, lhsT=wn_sb[:, kc2, :, mt * 128:(mt + 1) * 128],
                                         rhs=ot_sb[:, kc2], start=(kc2 == 0), stop=False,
                                         perf_mode=DR)
                    nc.tensor.matmul(pm, lhsT=wnb_sb[0:1, mt * 128:(mt + 1) * 128],
                                     rhs=ones_b[0:1, 0:FPC * O], start=False, stop=True)
                    nc.scalar.activation(msgn_sb[:, mt, :], pm, AF.Copy, scale=1.0 / 64.0)

            # ---------------- Phase A: 2 propagation steps over edges ----------------
            with tc.tile_pool(name="paps", bufs=4, space="PSUM") as paps, \
                 tc.tile_pool(name="papss", bufs=2, space="PSUM") as papss:
                for q in range(NQ):
                    xq = pa.tile([128, KC2, 2, 512], F8, tag="xq")
                    nc.sync.dma_start(out=xq, in_=e0t.ap()[q]
                                      .rearrange("(kc2 two p) n -> p kc2 two n", p=128, two=2))
                    um1t = pa1.tile([128, KC2, 2, 512], F8, tag="um1t")
                    for step in range(2):
                        rhs = xq if step == 0 else um1t
                        rscale = 1.0 if step == 0 else 1.0 / 16.0
                        bet_sb = bet64_sb if step == 0 else bet1k_sb
                        cst_e = cst_q if step == 0 else cst_1k
                        cst_n = cst_16 if step == 0 else ones_b
                        # --- a-wave: relu(X @ Wl1^T + bl1), transposed ---
                        relu_sb = pa1.tile([128, 8, 512], BF16, tag="relu")
                        for mt in range(8, 16):
                            pw_a = paps.tile([128, 512], F32, tag="wave")
                            for kc2 in range(KC2):
                                nc.tensor.matmul(pw_a,
                                                 lhsT=wcat_sb[:, kc2, :, mt * 128:(mt + 1) * 128],
                                                 rhs=rhs[:, kc2], start=(kc2 == 0),
                                                 stop=(kc2 == KC2 - 1), perf_mode=DR)
                            nc.scalar.activation(relu_sb[:, mt - 8, :], pw_a, AF.Relu,
                                                 bias=bl1t_sb[:, mt - 8:mt - 7], scale=rscale)
                        # --- logits + softmax over o (groups of 16) ---
                        pl = papss.tile([1, 512], F32, tag="pl")
                        for kc2 in range(8):
                            nc.tensor.matmul(pl, lhsT=wl2_sb[:, kc2:kc2 + 1],
                                             rhs=relu_sb[:, kc2, :], start=(kc2 == 0), stop=(kc2 == 7))
                        pl3 = pl.rearrange("o (g i) -> o g i", i=16)
                        mx = pa1.tile([1, 32], F32, tag="mx")
                        nc.vector.reduce_max(mx, pl3, axis=AX.X)
                        sub = pa1.tile([1, 512], F32, tag="sub")
                        nc.vector.tensor_tensor(sub.rearrange("o (g i) -> o g i", i=16), pl3,
                                                mx.broadcast_to((1, 32, 16)), op=ALU.subtract)
                        nc.scalar.activation(sub, sub, AF.Exp)
                        ex3 = sub.rearrange("o (g i) -> o g i", i=16)
                        sm = pa1.tile([1, 32], F32, tag="sm")
                        nc.vector.reduce_sum(sm, ex3, axis=AX.X)
                        rs = pa1.tile([1, 32], F32, tag="rs")
                        nc.vector.reciprocal(rs, sm)
                        w_sb = pa1.tile([1, 512], BF16, tag="w")
                        nc.vector.tensor_tensor(w_sb.rearrange("o (g i) -> o g i", i=16), ex3,
                                                rs.broadcast_to((1, 32, 16)), op=ALU.mult)
                        # --- msg_e wave; w-broadcast MMs emitted after 2 groups ---
                        e_ps = []
                        wb_e = pa1.tile([128, 512], F32, tag="wbe")
                        wb_n = pa1.tile([128, 512], F32, tag="wbn")
                        for mt in range(8):
                            pe = paps.tile([128, 512], F32, tag="wave")
                            for kc2 in range(KC2):
                                nc.tensor.matmul(pe,
                                                 lhsT=wcat_sb[:, kc2, :, mt * 128:(mt + 1) * 128],
                                                 rhs=rhs[:, kc2], start=(kc2 == 0),
                                                 stop=(kc2 == KC2 - 1), perf_mode=DR)
                            e_ps.append(pe)
                            if mt == 1:
                                # broadcast scaled w along partitions via K=1 matmuls
                                # (PE waits here on softmax, hidden under 2 MM groups)
                                pw_b = papss.tile([128, 512], F32, tag="pw")
                                nc.tensor.matmul(pw_b, lhsT=cst_e[0:1, 0:128], rhs=w_sb,
                                                 start=True, stop=True)
                                nc.scalar.copy(wb_e, pw_b)
                                pw_c = papss.tile([128, 512], F32, tag="pw")
                                nc.tensor.matmul(pw_c, lhsT=cst_n[0:1, 0:128], rhs=w_sb,
                                                 start=True, stop=True)
                                nc.scalar.copy(wb_n, pw_c)
                            if mt >= 1:
                                for cmt in ([0, 1] if mt == 1 else [mt]):
                                    _combine_e(nc, step, cmt, q, e_ps[cmt], wb_e, bet_sb,
                                               um1t, msum_b, pa1)
                        wb4 = wb_n.rearrange("p (f h o) -> p f h o", f=4, h=8)
                        # msg_n half (tiles 8..16): broadcast over h
                        for j in range(8):
                            mt = 8 + j
                            base = msgn_sb[:, j, q * 64:(q + 1) * 64]
                            mn_bc = bass.AP(tensor=base.tensor, offset=base.offset,
                                            ap=[list(base.ap[0]), [16, 4], [0, 8], [1, 16]])
                            if step == 0:
                                nc.vector.tensor_tensor(
                                    um1t[:, mt // 2, mt % 2, :]
                                    .rearrange("p (f h o) -> p f h o", f=4, h=8),
                                    mn_bc, wb4, op=ALU.mult)
                            else:
                                tmp = pa1.tile([128, 512], F32, tag="um2")
                                nc.vector.tensor_tensor(
                                    tmp.rearrange("p (f h o) -> p f h o", f=4, h=8),
                                    mn_bc, wb4, op=ALU.mult)
                                with nc.allow_low_precision(
                                        reason="msum store bf16; o-reduce of 16 terms"):
                                    nc.vector.reduce_sum(
                                        msum_b[:, mt, q * 32:(q + 1) * 32],
                                        tmp.rearrange("p (f h o) -> p f h o", f=4, h=8),
                                        axis=AX.X)
                    # ---- AG1 chunk q: gather this quad's msum cols from all cores ----
                    nc.sync.dma_start(out=ag1_ins[q].rearrange("(kc p) n -> p kc n", p=128),
                                      in_=msum_b[:, :, q * 32:(q + 1) * 32])
                    nc.gpsimd.collective_compute(
                        "AllGather", ALU.bypass, replica_groups=RG,
                        ins=[ag1_ins[q].opt()], outs=[ag1_outs[q].opt()])

        # ---------------- Phase B: human GRU, hidden-slice parallel ----------------
        with (
            tc.tile_pool(name="pbw", bufs=1) as pbw,
            tc.tile_pool(name="pbs", bufs=2) as pbs,
            tc.tile_pool(name="pb1", bufs=1) as pb1,
            tc.tile_pool(name="pbps", bufs=2, space="PSUM") as pbps,
            tc.tile_pool(name="pbps2", bufs=2, space="PSUM") as pbps2,
            tc.tile_pool(name="pcw", bufs=1) as pcw,
        ):
            wBh_sb = pbw.tile([128, KC, 3 * HS], BF16)
            nc.sync.dma_start(out=wBh_sb, in_=wBh_d.ap().rearrange("(kc p) m -> p kc m", p=128))
            wBi_sb = pbw.tile([128, KC, 3 * HS], BF16)
            nc.sync.dma_start(out=wBi_sb, in_=wBi_d.ap().rearrange("(kc p) m -> p kc m", p=128))
            bB_rz = pbw.tile([1, 2 * HS], BF16)
            nc.sync.dma_start(out=bB_rz, in_=bB_rz_d.ap())
            bB_in = pbw.tile([1, HS], BF16)
            nc.sync.dma_start(out=bB_in, in_=bB_in_d.ap())
            bB_hn = pbw.tile([1, HS], BF16)
            nc.sync.dma_start(out=bB_hn, in_=bB_hn_d.ap())
            pmat_sb = pbw.tile([128, RC, NFRAMES], BF16)
            nc.sync.dma_start(out=pmat_sb, in_=pmatd.ap().rearrange("(rc p) f -> p rc f", p=128))
            hrm_sb = pbw.tile([128, RC, HS], F32)
            nc.sync.dma_start(out=hrm_sb, in_=hrms.ap().rearrange("(rc p) m -> p rc m", p=128))
            hum_sb = pbw.tile([128, RC, HS], BF16)

            for rc in range(RC):
                ht_t = pbs.tile([128, KC, 128], BF16, tag="ht")
                nc.sync.dma_start(out=ht_t, in_=htg.ap()[:, rc * 128:(rc + 1) * 128]
                                  .rearrange("(kc p) n -> p kc n", p=128))
                ms_t = pbs.tile([128, KC, 128], BF16, tag="ms")
                for q in range(NQ):
                    nc.sync.dma_start(out=ms_t[:, :, q * 32:(q + 1) * 32],
                                      in_=ag1_outs[q][rc * D:(rc + 1) * D, :]
                                      .rearrange("(kc p) n -> p kc n", p=128))
                p_rz = pbps.tile([128, 2 * HS], F32, tag="rz")
                for kc in range(KC):
                    nc.tensor.matmul(p_rz, lhsT=ht_t[:, kc, :], rhs=wBh_sb[:, kc, 0:2 * HS],
                                     start=(kc == 0), stop=False)
                for kc in range(KC):
                    nc.tensor.matmul(p_rz, lhsT=ms_t[:, kc, :], rhs=wBi_sb[:, kc, 0:2 * HS],
                                     start=False, stop=False)
                nc.tensor.matmul(p_rz, lhsT=ones_b[0:1, 0:128], rhs=bB_rz[0:1, :],
                                 start=False, stop=True)
                p_hn = pbps.tile([128, HS], F32, tag="hn")
                for kc in range(KC):
                    nc.tensor.matmul(p_hn, lhsT=ht_t[:, kc, :], rhs=wBh_sb[:, kc, 2 * HS:3 * HS],
                                     start=(kc == 0), stop=False)
                nc.tensor.matmul(p_hn, lhsT=ones_b[0:1, 0:128], rhs=bB_hn[0:1, :],
                                 start=False, stop=True)
                p_in = pbps.tile([128, HS], F32, tag="in")
                for kc in range(KC):
                    nc.tensor.matmul(p_in, lhsT=ms_t[:, kc, :], rhs=wBi_sb[:, kc, 2 * HS:3 * HS],
                                     start=(kc == 0), stop=False)
                nc.tensor.matmul(p_in, lhsT=ones_b[0:1, 0:128], rhs=bB_in[0:1, :],
                                 start=False, stop=True)
                r_sb = pb1.tile([128, HS], F32, tag="r")
                nc.scalar.activation(r_sb, p_rz[:, 0:HS], AF.Sigmoid)
                z_sb = pb1.tile([128, HS], F32, tag="z")
                nc.scalar.activation(z_sb, p_rz[:, HS:2 * HS], AF.Sigmoid)
                t1 = pb1.tile([128, HS], F32, tag="t1")
                nc.vector.tensor_tensor(t1, r_sb, p_hn, op=ALU.mult)
                t2 = pb1.tile([128, HS], F32, tag="t2")
                nc.vector.tensor_tensor(t2, t1, p_in, op=ALU.add)
                n_sb = pb1.tile([128, HS], F32, tag="n")
                nc.scalar.activation(n_sb, t2, AF.Tanh)
                t3 = pb1.tile([128, HS], F32, tag="t3")
                nc.vector.tensor_tensor(t3, hrm_sb[:, rc, :], n_sb, op=ALU.subtract)
                t4 = pb1.tile([128, HS], F32, tag="t4")
                nc.vector.tensor_tensor(t4, z_sb, t3, op=ALU.mult)
                nc.vector.tensor_tensor(hum_sb[:, rc, :], n_sb, t4, op=ALU.add)

            # All_human^T slice: ah[mc] = hum[:, mc-chunk].T @ pmat   [256, 128]
            ahT_sb = pb1.tile([128, 2, NFRAMES], BF16, tag="ahT")
            for mc in range(2):
                pah = pbps2.tile([128, NFRAMES], F32, tag="pah")
                for rc in range(RC):
                    nc.tensor.matmul(pah, lhsT=hum_sb[:, rc, mc * 128:(mc + 1) * 128],
                                     rhs=pmat_sb[:, rc, :], start=(rc == 0), stop=(rc == RC - 1))
                nc.scalar.copy(ahT_sb[:, mc, :], pah)
            nc.sync.dma_start(out=ag2_in.rearrange("(mc p) f -> p mc f", p=128),
                              in_=ahT_sb)
            nc.gpsimd.collective_compute(
                "AllGather", ALU.bypass, replica_groups=RG,
                ins=[ag2_in.opt()], outs=[ag2_out.opt()])

            # -------- Phase C prefetched weights (loaded during B) --------
            wCh_sb = pcw.tile([128, KC, 3 * HS], BF16)
            nc.sync.dma_start(out=wCh_sb, in_=wCh_d.ap().rearrange("(kc p) m -> p kc m", p=128))
            wCi_sb = pcw.tile([128, KC, 3 * HS], BF16)
            nc.sync.dma_start(out=wCi_sb, in_=wCi_d.ap().rearrange("(kc p) m -> p kc m", p=128))
            scsf_sb = pcw.tile([128, KC, 2 * NFRAMES], BF16)
            nc.sync.dma_start(out=scsf_sb, in_=scsf_d.ap().rearrange("(kc p) n -> p kc n", p=128))
            bC_rz = pcw.tile([1, 2 * HS], BF16)
            nc.sync.dma_start(out=bC_rz, in_=bC_rz_d.ap())
            bC_in = pcw.tile([1, HS], BF16)
            nc.sync.dma_start(out=bC_in, in_=bC_in_d.ap())
            bC_hn = pcw.tile([1, HS], BF16)
            nc.sync.dma_start(out=bC_hn, in_=bC_hn_d.ap())
            sc4s_sb = pcw.tile([NFRAMES, HS], F32)
            nc.sync.dma_start(out=sc4s_sb, in_=sc4s_d.ap())
            sfs_sb = pcw.tile([NFRAMES, HS], F32)
            nc.sync.dma_start(out=sfs_sb, in_=sfs_d.ap())

        # ---------------- Phase C: two S-node GRUs, hidden-slice parallel ----------------
        with (
            tc.tile_pool(name="pc1", bufs=1) as pc1,
            tc.tile_pool(name="pcps", bufs=2, space="PSUM") as pcps,
            tc.tile_pool(name="pctps", bufs=2, space="PSUM") as pctps,
        ):
            ah_sb = pc1.tile([128, KC, NFRAMES], BF16)
            nc.sync.dma_start(out=ah_sb, in_=ag2_out
                              .rearrange("(kc p) f -> p kc f", p=128))

            # gh2 (vs S_f), biases folded in; stored for step 2
            g2h_rz = pc1.tile([NFRAMES, 2 * HS], BF16)
            p2h = pcps.tile([NFRAMES, 2 * HS], F32, tag="rz")
            for kc in range(KC):
                nc.tensor.matmul(p2h, lhsT=scsf_sb[:, kc, NFRAMES:2 * NFRAMES],
                                 rhs=wCh_sb[:, kc, 0:2 * HS], start=(kc == 0), stop=False)
            nc.tensor.matmul(p2h, lhsT=ones_b[0:1, 0:NFRAMES], rhs=bC_rz[0:1, :],
                             start=False, stop=True)
            nc.scalar.copy(g2h_rz, p2h)
            g2h_hn = pc1.tile([NFRAMES, HS], BF16)
            p2hn = pcps.tile([NFRAMES, HS], F32, tag="hn")
            for kc in range(KC):
                nc.tensor.matmul(p2hn, lhsT=scsf_sb[:, kc, NFRAMES:2 * NFRAMES],
                                 rhs=wCh_sb[:, kc, 2 * HS:3 * HS], start=(kc == 0), stop=False)
            nc.tensor.matmul(p2hn, lhsT=ones_b[0:1, 0:NFRAMES], rhs=bC_hn[0:1, :],
                             start=False, stop=True)
            nc.scalar.copy(g2h_hn, p2hn)

            # step-1 gates: gh1(S_C4) first (AG2-independent), then gi1(AH)
            p1_hn = pcps.tile([NFRAMES, HS], F32, tag="hn")
            for kc in range(KC):
                nc.tensor.matmul(p1_hn, lhsT=scsf_sb[:, kc, 0:NFRAMES],
                                 rhs=wCh_sb[:, kc, 2 * HS:3 * HS], start=(kc == 0), stop=False)
            nc.tensor.matmul(p1_hn, lhsT=ones_b[0:1, 0:NFRAMES], rhs=bC_hn[0:1, :],
                             start=False, stop=True)
            p1_rz = pcps.tile([NFRAMES, 2 * HS], F32, tag="rz")
            for kc in range(KC):
                nc.tensor.matmul(p1_rz, lhsT=scsf_sb[:, kc, 0:NFRAMES],
                                 rhs=wCh_sb[:, kc, 0:2 * HS], start=(kc == 0), stop=False)
            for kc in range(KC):
                nc.tensor.matmul(p1_rz, lhsT=ah_sb[:, kc, :], rhs=wCi_sb[:, kc, 0:2 * HS],
                                 start=False, stop=False)
            nc.tensor.matmul(p1_rz, lhsT=ones_b[0:1, 0:NFRAMES], rhs=bC_rz[0:1, :],
                             start=False, stop=True)
            p1_in = pcps.tile([NFRAMES, HS], F32, tag="in")
            for kc in range(KC):
                nc.tensor.matmul(p1_in, lhsT=ah_sb[:, kc, :], rhs=wCi_sb[:, kc, 2 * HS:3 * HS],
                                 start=(kc == 0), stop=False)
            nc.tensor.matmul(p1_in, lhsT=ones_b[0:1, 0:NFRAMES], rhs=bC_in[0:1, :],
                             start=False, stop=True)

            # step-1 elementwise -> s1 slice
            z1 = pc1.tile([NFRAMES, HS], F32, tag="z1")
            nc.scalar.activation(z1, p1_rz[:, HS:2 * HS], AF.Sigmoid)
            r1 = pc1.tile([NFRAMES, HS], F32, tag="r1")
            nc.scalar.activation(r1, p1_rz[:, 0:HS], AF.Sigmoid)
            u1 = pc1.tile([NFRAMES, HS], F32, tag="u1")
            nc.vector.tensor_tensor(u1, r1, p1_hn, op=ALU.mult)
            u2 = pc1.tile([NFRAMES, HS], F32, tag="u2")
            nc.vector.tensor_tensor(u2, u1, p1_in, op=ALU.add)
            n1 = pc1.tile([NFRAMES, HS], F32, tag="n1")
            nc.scalar.activation(n1, u2, AF.Tanh)
            u3 = pc1.tile([NFRAMES, HS], F32, tag="u3")
            nc.vector.tensor_tensor(u3, sc4s_sb, n1, op=ALU.subtract)
            u4 = pc1.tile([NFRAMES, HS], F32, tag="u4")
            nc.vector.tensor_tensor(u4, z1, u3, op=ALU.mult)
            s1_sb = pc1.tile([NFRAMES, HS], BF16, tag="s1")
            nc.vector.tensor_tensor(s1_sb, n1, u4, op=ALU.add)

            # transpose s1 slice -> [256, 128] and AllGather full s1^T
            s1t_sb = pc1.tile([128, 2, NFRAMES], BF16, tag="s1t")
            for mc in range(2):
                ptp = pctps.tile([128, NFRAMES], BF16, tag="tp")
                nc.tensor.transpose(ptp, s1_sb[:, mc * 128:(mc + 1) * 128], ident)
                nc.scalar.copy(s1t_sb[:, mc, :], ptp)
            nc.sync.dma_start(out=ag3_in.rearrange("(mc p) f -> p mc f", p=128),
                              in_=s1t_sb)
            nc.gpsimd.collective_compute(
                "AllGather", ALU.bypass, replica_groups=RG,
                ins=[ag3_in.opt()], outs=[ag3_out.opt()])
            s1t_g = pc1.tile([128, KC, NFRAMES], BF16)
            nc.sync.dma_start(out=s1t_g, in_=ag3_out
                              .rearrange("(kc p) f -> p kc f", p=128))

            # step-2 gates: gi2(s1); gh2 already in SBUF
            p2_rz = pcps.tile([NFRAMES, 2 * HS], F32, tag="rz")
            for kc in range(KC):
                nc.tensor.matmul(p2_rz, lhsT=s1t_g[:, kc, :], rhs=wCi_sb[:, kc, 0:2 * HS],
                                 start=(kc == 0), stop=(kc == KC - 1))
            p2_n = pcps.tile([NFRAMES, HS], F32, tag="in")
            for kc in range(KC):
                nc.tensor.matmul(p2_n, lhsT=s1t_g[:, kc, :], rhs=wCi_sb[:, kc, 2 * HS:3 * HS],
                                 start=(kc == 0), stop=False)
            nc.tensor.matmul(p2_n, lhsT=ones_b[0:1, 0:NFRAMES], rhs=bC_in[0:1, :],
                             start=False, stop=True)

            # step-2 elementwise -> out slice
            grz = pc1.tile([NFRAMES, 2 * HS], F32, tag="grz")
            nc.vector.tensor_tensor(grz, p2_rz, g2h_rz, op=ALU.add)
            z2 = pc1.tile([NFRAMES, HS], F32, tag="z2")
            nc.scalar.activation(z2, grz[:, HS:2 * HS], AF.Sigmoid)
            r2 = pc1.tile([NFRAMES, HS], F32, tag="r2")
            nc.scalar.activation(r2, grz[:, 0:HS], AF.Sigmoid)
            v1 = pc1.tile([NFRAMES, HS], F32, tag="v1")
            nc.vector.tensor_tensor(v1, r2, g2h_hn, op=ALU.mult)
            v2 = pc1.tile([NFRAMES, HS], F32, tag="v2")
            nc.vector.tensor_tensor(v2, v1, p2_n, op=ALU.add)
            n2 = pc1.tile([NFRAMES, HS], F32, tag="n2")
            nc.scalar.activation(n2, v2, AF.Tanh)
            v3 = pc1.tile([NFRAMES, HS], F32, tag="v3")
            nc.vector.tensor_tensor(v3, sfs_sb, n2, op=ALU.subtract)
            v4 = pc1.tile([NFRAMES, HS], F32, tag="v4")
            nc.vector.tensor_tensor(v4, z2, v3, op=ALU.mult)
            out_sb = pc1.tile([NFRAMES, HS], F32, tag="out")
            nc.vector.tensor_tensor(out_sb, n2, v4, op=ALU.add)
            nc.sync.dma_start(out=outp.ap(), in_=out_sb)

    nc.compile()
    return nc


def _prep_in_maps(inputs):
    E = np.ascontiguousarray(inputs["H_O_edges"].reshape(NFRAMES, ROWS, D))
    On = inputs["O_nodes"].reshape(NFRAMES, O, D)
    Hn = inputs["H_nodes"].reshape(NFRAMES, H, D)
    Sc4 = inputs["S_node_C4"].reshape(NFRAMES, D)
    Sf = np.ascontiguousarray(inputs["final_S_node"].transpose(0, 2, 1)).reshape(NFRAMES, D)
    Hn_rm = Hn.reshape(NR, D)  # rows = (frame, h)

    bB_rz_full = inputs["gh_bih"] + inputs["gh_bhh"]
    bC_rz_full = inputs["gs_bih"] + inputs["gs_bhh"]

    shared = {
        "wcat": np.ascontiguousarray(
            64.0 * np.concatenate([inputs["We"], inputs["Wl1"]], axis=0).T).astype(NF8),
        "bl1t": np.ascontiguousarray(64.0 * inputs["bl1"].reshape(8, 128).T).astype(np.float32),
        "bet64": np.ascontiguousarray(64.0 * inputs["be"].reshape(8, 128).T).astype(np.float32),
        "bet1k": np.ascontiguousarray(1024.0 * inputs["be"].reshape(8, 128).T).astype(np.float32),
        "wnt": np.ascontiguousarray(64.0 * inputs["Wn"].T).astype(NF8),
        "wnb": 64.0 * inputs["bn"][None, :].astype(NB),
        "wl2": np.ascontiguousarray(inputs["Wl2"][0].reshape(8, 128).T / 64.0).astype(NB),
        "htg": np.ascontiguousarray(Hn_rm.T).astype(NB),
        "pmat": np.ascontiguousarray(np.kron(np.eye(NFRAMES), np.ones((H, 1))) / H).astype(NB),
        "scsf": np.ascontiguousarray(np.concatenate([Sc4.T, Sf.T], axis=1)).astype(NB),
    }

    in_maps = []
    for c in range(NCORES):
        fr = slice(c * FPC, (c + 1) * FPC)
        Ec = E[fr]  # [16, 128, 2048]
        e0t = np.ascontiguousarray(
            Ec.reshape(NQ, 4, ROWS, D).transpose(0, 3, 1, 2).reshape(NQ, D, 512)).astype(NF8)
        ot = np.ascontiguousarray(
            On[fr].reshape(FPC * O, D).T).astype(NF8)
        hs = slice(c * HS, (c + 1) * HS)
        rows_rzn = np.r_[c * HS:(c + 1) * HS,
                         D + c * HS:D + (c + 1) * HS,
                         2 * D + c * HS:2 * D + (c + 1) * HS]
        rows_rz = rows_rzn[:2 * HS]
        m = dict(shared)
        m.update({
            "e0t": e0t,
            "ot": ot,
            "hrms": np.ascontiguousarray(Hn_rm[:, hs]).astype(np.float32),
            "wBi": np.ascontiguousarray((inputs["gh_wih"][rows_rzn] / float(O)).T).astype(NB),
            "wBh": np.ascontiguousarray(inputs["gh_whh"][rows_rzn].T).astype(NB),
            "bB_rz": bB_rz_full[rows_rz][None, :].astype(NB),
            "bB_in": inputs["gh_bih"][rows_rzn[2 * HS:]][None, :].astype(NB),
            "bB_hn": inputs["gh_bhh"][rows_rzn[2 * HS:]][None, :].astype(NB),
            "sc4s": np.ascontiguousarray(Sc4[:, hs]).astype(np.float32),
            "sfs": np.ascontiguousarray(Sf[:, hs]).astype(np.float32),
            "wCi": np.ascontiguousarray(inputs["gs_wih"][rows_rzn].T).astype(NB),
            "wCh": np.ascontiguousarray(inputs["gs_whh"][rows_rzn].T).astype(NB),
            "bC_rz": bC_rz_full[rows_rz][None, :].astype(NB),
            "bC_in": inputs["gs_bih"][rows_rzn[2 * HS:]][None, :].astype(NB),
            "bC_hn": inputs["gs_bhh"][rows_rzn[2 * HS:]][None, :].astype(NB),
        })
        in_maps.append(m)
    return in_maps


LAST_RESULT = None


def kernel(**inputs):
    global LAST_RESULT
    if "nc" not in _CACHE:
        _CACHE["nc"] = _build_nc()
    nc = _CACHE["nc"]
    in_maps = _prep_in_maps(inputs)
    trace = os.environ.get("KERNEL_TRACE", "0") == "1"
    res = bass_utils.run_bass_kernel_spmd(
        nc, in_maps, core_ids=list(range(NCORES)), trace=trace)
    LAST_RESULT = res
    out = np.empty((NFRAMES, D), np.float32)
    for c in range(NCORES):
        out[:, c * HS:(c + 1) * HS] = res.results[c]["outp"]
    return np.ascontiguousarray(out.reshape(B, F, D)).astype(np.float32)


if __name__ == "__main__":
    np.random.seed(0)
    ins = {
        "S_node_C4": np.random.randn(B, F, D).astype(np.float32),
        "final_S_node": np.random.randn(B, D, F).astype(np.float32),
        "H_nodes": np.random.randn(B, F, H, D).astype(np.float32),
        "O_nodes": np.random.randn(B, F, O, D).astype(np.float32),
        "H_O_edges": np.random.randn(B, F, H, O, D).astype(np.float32),
        "Wn": np.random.randn(D // 2, D).astype(np.float32) * 0.02,
        "bn": np.random.randn(D // 2).astype(np.float32) * 0.02,
        "We": np.random.randn(D // 2, D).astype(np.float32) * 0.02,
        "be": np.random.randn(D // 2).astype(np.float32) * 0.02,
        "Wl1": np.random.randn(D // 2, D).astype(np.float32) * 0.02,
        "bl1": np.random.randn(D // 2).astype(np.float32) * 0.02,
        "Wl2": np.random.randn(1, D // 2).astype(np.float32) * 0.02,
        "bl2": np.random.randn(1).astype(np.float32) * 0.02,
        "gh_wih": np.random.randn(3 * D, D).astype(np.float32) * 0.02,
        "gh_whh": np.random.randn(3 * D, D).astype(np.float32) * 0.02,
        "gh_bih": np.random.randn(3 * D).astype(np.float32) * 0.02,
        "gh_bhh": np.random.randn(3 * D).astype(np.float32) * 0.02,
        "gs_wih": np.random.randn(3 * D, D).astype(np.float32) * 0.02,
        "gs_whh": np.random.randn(3 * D, D).astype(np.float32) * 0.02,
        "gs_bih": np.random.randn(3 * D).astype(np.float32) * 0.02,
        "gs_bhh": np.random.randn(3 * D).astype(np.float32) * 0.02,
    }
    out = kernel(**ins)
    print("kernel ran, out shape", out.shape, out.dtype, float(np.abs(out).mean()))


# revision 25
# speedup vs baseline: 103.6577x; 103.6577x over previous
"""Trainium2 Bass kernel for nn_Graph_Enhance_model (GNN message passing).

Self-contained: hardcodes shapes B=4,F=32,H=8,O=16,D=2048, 8 cores.
Phase A (edge waves): data-parallel over the 128 (b,f) frames, 16/core.
Phases B/C (GRUs): tensor-parallel over the hidden dim (256 hidden
units per core, 768 of the 6144 gate rows), stitched with AllGathers.
"""

import os
import sys

for _p in ("/opt/trn_rl_repo", "/opt/pypackages"):
    if _p not in sys.path and os.path.isdir(_p):
        sys.path.append(_p)

import numpy as np
import ml_dtypes

import concourse.bass as bass
import concourse.bacc as bacc
import concourse.tile as tile
import concourse.mybir as mybir
from concourse import bass_utils
from concourse.masks import make_identity

BF16 = mybir.dt.bfloat16
F32 = mybir.dt.float32
AF = mybir.ActivationFunctionType
ALU = mybir.AluOpType
AX = mybir.AxisListType

NB = ml_dtypes.bfloat16

B, F, H, O, D = 4, 32, 8, 16, 2048
NFRAMES = B * F          # 128
NCORES = 8
FPC = NFRAMES // NCORES  # 16 frames per core (phase A)
ROWS = H * O             # 128 edge rows per frame
KC = D // 128            # 16 K-chunks
NQ = FPC // 4            # 4 quads of 4 frames
HS = D // NCORES         # 256 hidden units per core (phases B/C)
NR = NFRAMES * H         # 1024 human rows globally
RC = NR // 128           # 8 row-chunks of 128

_CACHE = {}


def _combine_e(nc, step, mt, q, pe, wb_sb, bet_sb, um1t, msum_b, pool):
    """UM = (msg_e_psum + be) * w ; step 1 also reduces over o into msum."""
    if step == 0:
        nc.vector.scalar_tensor_tensor(out=um1t[:, mt, :], in0=pe,
                                       scalar=bet_sb[:, mt:mt + 1], in1=wb_sb,
                                       op0=ALU.add, op1=ALU.mult)
    else:
        tmp = pool.tile([128, 512], F32, tag="um2")
        nc.vector.scalar_tensor_tensor(out=tmp, in0=pe,
                                       scalar=bet_sb[:, mt:mt + 1], in1=wb_sb,
                                       op0=ALU.add, op1=ALU.mult)
        with nc.allow_low_precision(reason="msum store bf16; o-reduce of 16 terms"):
            nc.vector.reduce_sum(msum_b[:, mt, q * 32:(q + 1) * 32],
                                 tmp.rearrange("p (f h o) -> p f h o", f=4, h=8),
                                 axis=AX.X)


def _build_nc():
    nc = bacc.Bacc("TRN2", target_bir_lowering=False, debug=False, num_devices=NCORES)

    dt_in = {}

    def din(name, shape, dt):
        dt_in[name] = nc.dram_tensor(name, shape, dt, kind="ExternalInput")
        return dt_in[name]

    # ---- phase A inputs (per-core frames) ----
    e0t = din("e0t", [NQ, D, 512], BF16)
    ot = din("ot", [D, FPC * O], BF16)
    wcat = din("wcat", [D, D], BF16)
    bl1td = din("bl1t", [128, 8], F32)
    betd = din("bet", [128, 8], F32)
    wnt = din("wnt", [D, D // 2], BF16)
    wnb = din("wnb", [1, D // 2], BF16)
    wl2 = din("wl2", [128, 8], BF16)
    # ---- phase B inputs (all frames; per-core hidden slice) ----
    htg = din("htg", [D, NR], BF16)          # H_nodes^T, all 128 frames
    pmatd = din("pmat", [NR, NFRAMES], BF16)  # mean-over-H matrix /8
    hrms = din("hrms", [NR, HS], F32)        # H_nodes rows, hidden slice
    wBi_d = din("wBi", [D, 3 * HS], BF16)    # (gh_wih/16)[rzn slice]^T
    wBh_d = din("wBh", [D, 3 * HS], BF16)    # gh_whh[rzn slice]^T
    bB_rz_d = din("bB_rz", [1, 2 * HS], BF16)
    bB_in_d = din("bB_in", [1, HS], BF16)
    bB_hn_d = din("bB_hn", [1, HS], BF16)
    # ---- phase C inputs ----
    scsf_d = din("scsf", [D, 2 * NFRAMES], BF16)   # [S_C4^T | S_f^T]
    sc4s_d = din("sc4s", [NFRAMES, HS], F32)
    sfs_d = din("sfs", [NFRAMES, HS], F32)
    wCi_d = din("wCi", [D, 3 * HS], BF16)
    wCh_d = din("wCh", [D, 3 * HS], BF16)
    bC_rz_d = din("bC_rz", [1, 2 * HS], BF16)      # (bih+bhh)[rz]
    bC_in_d = din("bC_in", [1, HS], BF16)
    bC_hn_d = din("bC_hn", [1, HS], BF16)

    outp = nc.dram_tensor("outp", [NFRAMES, HS], F32, kind="ExternalOutput")

    from contextlib import ExitStack

    RG = [list(range(NCORES))]

    with tile.TileContext(nc) as tc, ExitStack() as ctx:
        glob = ctx.enter_context(tc.tile_pool(name="glob", bufs=1))
        dram = ctx.enter_context(tc.tile_pool(name="dram", bufs=1, space="DRAM"))

        ag1_ins = [dram.tile([D, 32], BF16, name=f"ag1i{q}") for q in range(NQ)]
        ag1_outs = [dram.tile([NCORES * D, 32], BF16, addr_space="Shared",
                              name=f"ag1o{q}") for q in range(NQ)]
        ag2_in = dram.tile([2 * 128, NFRAMES], BF16)
        ag2_out = dram.tile([D, NFRAMES], BF16, addr_space="Shared")
        ag3_in = dram.tile([2 * 128, NFRAMES], BF16)
        ag3_out = dram.tile([D, NFRAMES], BF16, addr_space="Shared")

        ones_b = glob.tile([1, 512], BF16)
        nc.vector.memset(ones_b, 1.0)
        ident = glob.tile([128, 128], BF16)
        make_identity(nc, ident)

        wl2_sb = glob.tile([128, 8], BF16)
        nc.sync.dma_start(out=wl2_sb, in_=wl2.ap())
        bl1t_sb = glob.tile([128, 8], F32)
        nc.sync.dma_start(out=bl1t_sb, in_=bl1td.ap())
        bet_sb = glob.tile([128, 8], F32)
        nc.sync.dma_start(out=bet_sb, in_=betd.ap())

        msgn_sb = glob.tile([128, 8, FPC * O], BF16)    # [1024, 256] transposed msg_n
        msum_b = glob.tile([128, KC, FPC * H], BF16)    # M_sum2^T (raw sum over o)

        with (
            tc.tile_pool(name="pwcat", bufs=1) as pwcat,
            tc.tile_pool(name="pa", bufs=2) as pa,
            tc.tile_pool(name="pa1", bufs=1) as pa1,
        ):
            wcat_sb = pwcat.tile([128, KC, D], BF16)
            nc.sync.dma_start(out=wcat_sb, in_=wcat.ap().rearrange("(kc p) m -> p kc m", p=128))

            # ---------------- Phase 0: msg_n^T = Wn @ O^T + bn ----------------
            with (
                tc.tile_pool(name="p0", bufs=1) as p0,
                tc.tile_pool(name="p0ps", bufs=4, space="PSUM") as p0ps,
            ):
                wnb_sb = p0.tile([1, D // 2], BF16)
                nc.sync.dma_start(out=wnb_sb, in_=wnb.ap())
                ot_sb = p0.tile([128, KC, FPC * O], BF16)
                nc.sync.dma_start(out=ot_sb, in_=ot.ap().rearrange("(kc p) n -> p kc n", p=128))
                for half in range(2):
                    wn_sb = p0.tile([128, KC, 512], BF16, tag="wn")
                    nc.sync.dma_start(out=wn_sb, in_=wnt.ap()[:, half * 512:(half + 1) * 512]
                                      .rearrange("(kc p) m -> p kc m", p=128))
                    for mt4 in range(4):
                        mt = half * 4 + mt4
                        pm = p0ps.tile([128, FPC * O], F32, tag="pm")
                        for kc in range(KC):
                            nc.tensor.matmul(pm, lhsT=wn_sb[:, kc, mt4 * 128:(mt4 + 1) * 128],
                                             rhs=ot_sb[:, kc, :], start=(kc == 0), stop=False)
                        nc.tensor.matmul(pm, lhsT=wnb_sb[0:1, mt * 128:(mt + 1) * 128],
                                         rhs=ones_b[0:1, 0:FPC * O], start=False, stop=True)
                        nc.scalar.copy(msgn_sb[:, mt, :], pm)

            # ---------------- Phase A: 2 propagation steps over edges ----------------
            with tc.tile_pool(name="paps", bufs=4, space="PSUM") as paps, \
                 tc.tile_pool(name="papss", bufs=2, space="PSUM") as papss:
                for q in range(NQ):
                    xq = pa.tile([128, KC, 512], BF16, tag="xq")
                    nc.sync.dma_start(out=xq, in_=e0t.ap()[q].rearrange("(kc p) n -> p kc n", p=128))
                    um1t = pa1.tile([128, KC, 512], BF16, tag="um1t")
                    for step in range(2):
                        rhs = xq if step == 0 else um1t
                        # --- a-wave: relu(X @ Wl1^T + bl1), transposed ---
                        relu_sb = pa1.tile([128, 8, 512], BF16, tag="relu")
                        for mt in range(8, 16):
                            pw_a = paps.tile([128, 512], F32, tag="wave")
                            for kc in range(KC):
                                nc.tensor.matmul(pw_a, lhsT=wcat_sb[:, kc, mt * 128:(mt + 1) * 128],
                                                 rhs=rhs[:, kc, :], start=(kc == 0), stop=(kc == KC - 1))
                            nc.scalar.activation(relu_sb[:, mt - 8, :], pw_a, AF.Relu,
                                                 bias=bl1t_sb[:, mt - 8:mt - 7])
                        # --- logits + softmax over o (groups of 16) ---
                        pl = papss.tile([1, 512], F32, tag="pl")
                        for kc2 in range(8):
                            nc.tensor.matmul(pl, lhsT=wl2_sb[:, kc2:kc2 + 1],
                                             rhs=relu_sb[:, kc2, :], start=(kc2 == 0), stop=(kc2 == 7))
                        pl3 = pl.rearrange("o (g i) -> o g i", i=16)
                        mx = pa1.tile([1, 32], F32, tag="mx")
                        nc.vector.reduce_max(mx, pl3, axis=AX.X)
                        sub = pa1.tile([1, 512], F32, tag="sub")
                        nc.vector.tensor_tensor(sub.rearrange("o (g i) -> o g i", i=16), pl3,
                                                mx.broadcast_to((1, 32, 16)), op=ALU.subtract)
                        nc.scalar.activation(sub, sub, AF.Exp)
                        ex3 = sub.rearrange("o (g i) -> o g i", i=16)
                        sm = pa1.tile([1, 32], F32, tag="sm")
                        nc.vector.reduce_sum(sm, ex3, axis=AX.X)
                        rs = pa1.tile([1, 32], F32, tag="rs")
                        nc.vector.reciprocal(rs, sm)
                        w_sb = pa1.tile([1, 512], BF16, tag="w")
                        nc.vector.tensor_tensor(w_sb.rearrange("o (g i) -> o g i", i=16), ex3,
                                                rs.broadcast_to((1, 32, 16)), op=ALU.mult)
                        # --- msg_e wave; w-broadcast MM emitted after 2 groups ---
                        e_ps = []
                        wb_sb = pa1.tile([128, 512], F32, tag="wb")
                        for mt in range(8):
                            pe = paps.tile([128, 512], F32, tag="wave")
                            for kc in range(KC):
                                nc.tensor.matmul(pe, lhsT=wcat_sb[:, kc, mt * 128:(mt + 1) * 128],
                                                 rhs=rhs[:, kc, :], start=(kc == 0), stop=(kc == KC - 1))
                            e_ps.append(pe)
                            if mt == 1:
                                # broadcast w along partitions via K=1 matmul (PE waits
                                # here on softmax, hidden under the first 2 MM groups)
                                pw_b = papss.tile([128, 512], F32, tag="pw")
                                nc.tensor.matmul(pw_b, lhsT=ones_b[0:1, 0:128], rhs=w_sb,
                                                 start=True, stop=True)
                                nc.scalar.copy(wb_sb, pw_b)
                            if mt >= 1:
                                for cmt in ([0, 1] if mt == 1 else [mt]):
                                    _combine_e(nc, step, cmt, q, e_ps[cmt], wb_sb, bet_sb,
                                               um1t, msum_b, pa1)
                        wb4 = wb_sb.rearrange("p (f h o) -> p f h o", f=4, h=8)
                        # msg_n half (tiles 8..16): broadcast over h
                        for j in range(8):
                            mt = 8 + j
                            base = msgn_sb[:, j, q * 64:(q + 1) * 64]
                            mn_bc = bass.AP(tensor=base.tensor, offset=base.offset,
                                            ap=[list(base.ap[0]), [16, 4], [0, 8], [1, 16]])
                            if step == 0:
                                nc.vector.tensor_tensor(
                                    um1t[:, mt, :].rearrange("p (f h o) -> p f h o", f=4, h=8),
                                    mn_bc, wb4, op=ALU.mult)
                            else:
                                tmp = pa1.tile([128, 512], F32, tag="um2")
                                nc.vector.tensor_tensor(
                                    tmp.rearrange("p (f h o) -> p f h o", f=4, h=8),
                                    mn_bc, wb4, op=ALU.mult)
                                with nc.allow_low_precision(
                                        reason="msum store bf16; o-reduce of 16 terms"):
                                    nc.vector.reduce_sum(
                                        msum_b[:, mt, q * 32:(q + 1) * 32],
                                        tmp.rearrange("p (f h o) -> p f h o", f=4, h=8),
                                        axis=AX.X)
                    # ---- AG1 chunk q: gather this quad's msum cols from all cores ----
                    if step == 1:
                        nc.sync.dma_start(out=ag1_ins[q].rearrange("(kc p) n -> p kc n", p=128),
                                          in_=msum_b[:, :, q * 32:(q + 1) * 32])
                        nc.gpsimd.collective_compute(
                            "AllGather", ALU.bypass, replica_groups=RG,
                            ins=[ag1_ins[q].opt()], outs=[ag1_outs[q].opt()])


        # ---------------- Phase B: human GRU, hidden-slice parallel ----------------
        with (
            tc.tile_pool(name="pbw", bufs=1) as pbw,
            tc.tile_pool(name="pbs", bufs=2) as pbs,
            tc.tile_pool(name="pb1", bufs=1) as pb1,
            tc.tile_pool(name="pbps", bufs=2, space="PSUM") as pbps,
            tc.tile_pool(name="pbps2", bufs=2, space="PSUM") as pbps2,
            tc.tile_pool(name="pcw", bufs=1) as pcw,
        ):
            wBh_sb = pbw.tile([128, KC, 3 * HS], BF16)
            nc.sync.dma_start(out=wBh_sb, in_=wBh_d.ap().rearrange("(kc p) m -> p kc m", p=128))
            wBi_sb = pbw.tile([128, KC, 3 * HS], BF16)
            nc.sync.dma_start(out=wBi_sb, in_=wBi_d.ap().rearrange("(kc p) m -> p kc m", p=128))
            bB_rz = pbw.tile([1, 2 * HS], BF16)
            nc.sync.dma_start(out=bB_rz, in_=bB_rz_d.ap())
            bB_in = pbw.tile([1, HS], BF16)
            nc.sync.dma_start(out=bB_in, in_=bB_in_d.ap())
            bB_hn = pbw.tile([1, HS], BF16)
            nc.sync.dma_start(out=bB_hn, in_=bB_hn_d.ap())
            pmat_sb = pbw.tile([128, RC, NFRAMES], BF16)
            nc.sync.dma_start(out=pmat_sb, in_=pmatd.ap().rearrange("(rc p) f -> p rc f", p=128))
            hrm_sb = pbw.tile([128, RC, HS], F32)
            nc.sync.dma_start(out=hrm_sb, in_=hrms.ap().rearrange("(rc p) m -> p rc m", p=128))
            hum_sb = pbw.tile([128, RC, HS], BF16)

            for rc in range(RC):
                ht_t = pbs.tile([128, KC, 128], BF16, tag="ht")
                nc.sync.dma_start(out=ht_t, in_=htg.ap()[:, rc * 128:(rc + 1) * 128]
                                  .rearrange("(kc p) n -> p kc n", p=128))
                ms_t = pbs.tile([128, KC, 128], BF16, tag="ms")
                for q in range(NQ):
                    nc.sync.dma_start(out=ms_t[:, :, q * 32:(q + 1) * 32],
                                      in_=ag1_outs[q][rc * D:(rc + 1) * D, :]
                                      .rearrange("(kc p) n -> p kc n", p=128))
                p_rz = pbps.tile([128, 2 * HS], F32, tag="rz")
                for kc in range(KC):
                    nc.tensor.matmul(p_rz, lhsT=ht_t[:, kc, :], rhs=wBh_sb[:, kc, 0:2 * HS],
                                     start=(kc == 0), stop=False)
                for kc in range(KC):
                    nc.tensor.matmul(p_rz, lhsT=ms_t[:, kc, :], rhs=wBi_sb[:, kc, 0:2 * HS],
                                     start=False, stop=False)
                nc.tensor.matmul(p_rz, lhsT=ones_b[0:1, 0:128], rhs=bB_rz[0:1, :],
                                 start=False, stop=True)
                p_hn = pbps.tile([128, HS], F32, tag="hn")
                for kc in range(KC):
                    nc.tensor.matmul(p_hn, lhsT=ht_t[:, kc, :], rhs=wBh_sb[:, kc, 2 * HS:3 * HS],
                                     start=(kc == 0), stop=False)
                nc.tensor.matmul(p_hn, lhsT=ones_b[0:1, 0:128], rhs=bB_hn[0:1, :],
                                 start=False, stop=True)
                p_in = pbps.tile([128, HS], F32, tag="in")
                for kc in range(KC):
                    nc.tensor.matmul(p_in, lhsT=ms_t[:, kc, :], rhs=wBi_sb[:, kc, 2 * HS:3 * HS],
                                     start=(kc == 0), stop=False)
                nc.tensor.matmul(p_in, lhsT=ones_b[0:1, 0:128], rhs=bB_in[0:1, :],
                                 start=False, stop=True)
                r_sb = pb1.tile([128, HS], F32, tag="r")
                nc.scalar.activation(r_sb, p_rz[:, 0:HS], AF.Sigmoid)
                z_sb = pb1.tile([128, HS], F32, tag="z")
                nc.scalar.activation(z_sb, p_rz[:, HS:2 * HS], AF.Sigmoid)
                t1 = pb1.tile([128, HS], F32, tag="t1")
                nc.vector.tensor_tensor(t1, r_sb, p_hn, op=ALU.mult)
                t2 = pb1.tile([128, HS], F32, tag="t2")
                nc.vector.tensor_tensor(t2, t1, p_in, op=ALU.add)
                n_sb = pb1.tile([128, HS], F32, tag="n")
                nc.scalar.activation(n_sb, t2, AF.Tanh)
                t3 = pb1.tile([128, HS], F32, tag="t3")
                nc.vector.tensor_tensor(t3, hrm_sb[:, rc, :], n_sb, op=ALU.subtract)
                t4 = pb1.tile([128, HS], F32, tag="t4")
                nc.vector.tensor_tensor(t4, z_sb, t3, op=ALU.mult)
                nc.vector.tensor_tensor(hum_sb[:, rc, :], n_sb, t4, op=ALU.add)

            # All_human^T slice: ah[mc] = hum[:, mc-chunk].T @ pmat   [256, 128]
            ahT_sb = pb1.tile([128, 2, NFRAMES], BF16, tag="ahT")
            for mc in range(2):
                pah = pbps2.tile([128, NFRAMES], F32, tag="pah")
                for rc in range(RC):
                    nc.tensor.matmul(pah, lhsT=hum_sb[:, rc, mc * 128:(mc + 1) * 128],
                                     rhs=pmat_sb[:, rc, :], start=(rc == 0), stop=(rc == RC - 1))
                nc.scalar.copy(ahT_sb[:, mc, :], pah)
            nc.sync.dma_start(out=ag2_in.rearrange("(mc p) f -> p mc f", p=128),
                              in_=ahT_sb)
            nc.gpsimd.collective_compute(
                "AllGather", ALU.bypass, replica_groups=RG,
                ins=[ag2_in.opt()], outs=[ag2_out.opt()])

            # -------- Phase C prefetched weights (loaded during B) --------
            wCh_sb = pcw.tile([128, KC, 3 * HS], BF16)
            nc.sync.dma_start(out=wCh_sb, in_=wCh_d.ap().rearrange("(kc p) m -> p kc m", p=128))
            wCi_sb = pcw.tile([128, KC, 3 * HS], BF16)
            nc.sync.dma_start(out=wCi_sb, in_=wCi_d.ap().rearrange("(kc p) m -> p kc m", p=128))
            scsf_sb = pcw.tile([128, KC, 2 * NFRAMES], BF16)
            nc.sync.dma_start(out=scsf_sb, in_=scsf_d.ap().rearrange("(kc p) n -> p kc n", p=128))
            bC_rz = pcw.tile([1, 2 * HS], BF16)
            nc.sync.dma_start(out=bC_rz, in_=bC_rz_d.ap())
            bC_in = pcw.tile([1, HS], BF16)
            nc.sync.dma_start(out=bC_in, in_=bC_in_d.ap())
            bC_hn = pcw.tile([1, HS], BF16)
            nc.sync.dma_start(out=bC_hn, in_=bC_hn_d.ap())
            sc4s_sb = pcw.tile([NFRAMES, HS], F32)
            nc.sync.dma_start(out=sc4s_sb, in_=sc4s_d.ap())
            sfs_sb = pcw.tile([NFRAMES, HS], F32)
            nc.sync.dma_start(out=sfs_sb, in_=sfs_d.ap())

        # ---------------- Phase C: two S-node GRUs, hidden-slice parallel ----------------
        with (
            tc.tile_pool(name="pc1", bufs=1) as pc1,
            tc.tile_pool(name="pcps", bufs=2, space="PSUM") as pcps,
            tc.tile_pool(name="pctps", bufs=2, space="PSUM") as pctps,
        ):
            ah_sb = pc1.tile([128, KC, NFRAMES], BF16)
            nc.sync.dma_start(out=ah_sb, in_=ag2_out
                              .rearrange("(kc p) f -> p kc f", p=128))

            # gh2 (vs S_f), biases folded in; stored for step 2
            g2h_rz = pc1.tile([NFRAMES, 2 * HS], BF16)
            p2h = pcps.tile([NFRAMES, 2 * HS], F32, tag="rz")
            for kc in range(KC):
                nc.tensor.matmul(p2h, lhsT=scsf_sb[:, kc, NFRAMES:2 * NFRAMES],
                                 rhs=wCh_sb[:, kc, 0:2 * HS], start=(kc == 0), stop=False)
            nc.tensor.matmul(p2h, lhsT=ones_b[0:1, 0:NFRAMES], rhs=bC_rz[0:1, :],
                             start=False, stop=True)
            nc.scalar.copy(g2h_rz, p2h)
            g2h_hn = pc1.tile([NFRAMES, HS], BF16)
            p2hn = pcps.tile([NFRAMES, HS], F32, tag="hn")
            for kc in range(KC):
                nc.tensor.matmul(p2hn, lhsT=scsf_sb[:, kc, NFRAMES:2 * NFRAMES],
                                 rhs=wCh_sb[:, kc, 2 * HS:3 * HS], start=(kc == 0), stop=False)
            nc.tensor.matmul(p2hn, lhsT=ones_b[0:1, 0:NFRAMES], rhs=bC_hn[0:1, :],
                             start=False, stop=True)
            nc.scalar.copy(g2h_hn, p2hn)

            # step-1 gates: gh1(S_C4) first (AG2-independent), then gi1(AH)
            p1_hn = pcps.tile([NFRAMES, HS], F32, tag="hn")
            for kc in range(KC):
                nc.tensor.matmul(p1_hn, lhsT=scsf_sb[:, kc, 0:NFRAMES],
                                 rhs=wCh_sb[:, kc, 2 * HS:3 * HS], start=(kc == 0), stop=False)
            nc.tensor.matmul(p1_hn, lhsT=ones_b[0:1, 0:NFRAMES], rhs=bC_hn[0:1, :],
                             start=False, stop=True)
            p1_rz = pcps.tile([NFRAMES, 2 * HS], F32, tag="rz")
            for kc in range(KC):
                nc.tensor.matmul(p1_rz, lhsT=scsf_sb[:, kc, 0:NFRAMES],
                                 rhs=wCh_sb[:, kc, 0:2 * HS], start=(kc == 0), stop=False)
            for kc in range(KC):
                nc.tensor.matmul(p1_rz, lhsT=ah_sb[:, kc, :], rhs=wCi_sb[:, kc, 0:2 * HS],
                                 start=False, stop=False)
            nc.tensor.matmul(p1_rz, lhsT=ones_b[0:1, 0:NFRAMES], rhs=bC_rz[0:1, :],
                             start=False, stop=True)
            p1_in = pcps.tile([NFRAMES, HS], F32, tag="in")
            for kc in range(KC):
                nc.tensor.matmul(p1_in, lhsT=ah_sb[:, kc, :], rhs=wCi_sb[:, kc, 2 * HS:3 * HS],
                                 start=(kc == 0), stop=False)
            nc.tensor.matmul(p1_in, lhsT=ones_b[0:1, 0:NFRAMES], rhs=bC_in[0:1, :],
                             start=False, stop=True)

            # step-1 elementwise -> s1 slice
            z1 = pc1.tile([NFRAMES, HS], F32, tag="z1")
            nc.scalar.activation(z1, p1_rz[:, HS:2 * HS], AF.Sigmoid)
            r1 = pc1.tile([NFRAMES, HS], F32, tag="r1")
            nc.scalar.activation(r1, p1_rz[:, 0:HS], AF.Sigmoid)
            u1 = pc1.tile([NFRAMES, HS], F32, tag="u1")
            nc.vector.tensor_tensor(u1, r1, p1_hn, op=ALU.mult)
            u2 = pc1.tile([NFRAMES, HS], F32, tag="u2")
            nc.vector.tensor_tensor(u2, u1, p1_in, op=ALU.add)
            n1 = pc1.tile([NFRAMES, HS], F32, tag="n1")
            nc.scalar.activation(n1, u2, AF.Tanh)
            u3 = pc1.tile([NFRAMES, HS], F32, tag="u3")
            nc.vector.tensor_tensor(u3, sc4s_sb, n1, op=ALU.subtract)
            u4 = pc1.tile([NFRAMES, HS], F32, tag="u4")
            nc.vector.tensor_tensor(u4, z1, u3, op=ALU.mult)
            s1_sb = pc1.tile([NFRAMES, HS], BF16, tag="s1")
            nc.vector.tensor_tensor(s1_sb, n1, u4, op=ALU.add)

            # transpose s1 slice -> [256, 128] and AllGather full s1^T
            s1t_sb = pc1.tile([128, 2, NFRAMES], BF16, tag="s1t")
            for mc in range(2):
                ptp = pctps.tile([128, NFRAMES], BF16, tag="tp")
                nc.tensor.transpose(ptp, s1_sb[:, mc * 128:(mc + 1) * 128], ident)
                nc.scalar.copy(s1t_sb[:, mc, :], ptp)
            nc.sync.dma_start(out=ag3_in.rearrange("(mc p) f -> p mc f", p=128),
                              in_=s1t_sb)
            nc.gpsimd.collective_compute(
                "AllGather", ALU.bypass, replica_groups=RG,
                ins=[ag3_in.opt()], outs=[ag3_out.opt()])
            s1t_g = pc1.tile([128, KC, NFRAMES], BF16)
            nc.sync.dma_start(out=s1t_g, in_=ag3_out
                              .rearrange("(kc p) f -> p kc f", p=128))

            # step-2 gates: gi2(s1); gh2 already in SBUF
            p2_rz = pcps.tile([NFRAMES, 2 * HS], F32, tag="rz")
            for kc in range(KC):
                nc.tensor.matmul(p2_rz, lhsT=s1t_g[:, kc, :], rhs=wCi_sb[:, kc, 0:2 * HS],
                                 start=(kc == 0), stop=(kc == KC - 1))
            p2_n = pcps.tile([NFRAMES, HS], F32, tag="in")
            for kc in range(KC):
                nc.tensor.matmul(p2_n, lhsT=s1t_g[:, kc, :], rhs=wCi_sb[:, kc, 2 * HS:3 * HS],
                                 start=(kc == 0), stop=False)
            nc.tensor.matmul(p2_n, lhsT=ones_b[0:1, 0:NFRAMES], rhs=bC_in[0:1, :],
                             start=False, stop=True)

            # step-2 elementwise -> out slice
            grz = pc1.tile([NFRAMES, 2 * HS], F32, tag="grz")
            nc.vector.tensor_tensor(grz, p2_rz, g2h_rz, op=ALU.add)
            z2 = pc1.tile([NFRAMES, HS], F32, tag="z2")
            nc.scalar.activation(z2, grz[:, HS:2 * HS], AF.Sigmoid)
            r2 = pc1.tile([NFRAMES, HS], F32, tag="r2")
            nc.scalar.activation(r2, grz[:, 0:HS], AF.Sigmoid)
            v1 = pc1.tile([NFRAMES, HS], F32, tag="v1")
            nc.vector.tensor_tensor(v1, r2, g2h_hn, op=ALU.mult)
            v2 = pc1.tile([NFRAMES, HS], F32, tag="v2")
            nc.vector.tensor_tensor(v2, v1, p2_n, op=ALU.add)
            n2 = pc1.tile([NFRAMES, HS], F32, tag="n2")
            nc.scalar.activation(n2, v2, AF.Tanh)
            v3 = pc1.tile([NFRAMES, HS], F32, tag="v3")
            nc.vector.tensor_tensor(v3, sfs_sb, n2, op=ALU.subtract)
            v4 = pc1.tile([NFRAMES, HS], F32, tag="v4")
            nc.vector.tensor_tensor(v4, z2, v3, op=ALU.mult)
            out_sb = pc1.tile([NFRAMES, HS], F32, tag="out")
            nc.vector.tensor_tensor(out_sb, n2, v4, op=ALU.add)
            nc.sync.dma_start(out=outp.ap(), in_=out_sb)

    nc.compile()
    return nc


def _prep_in_maps(inputs):
    E = np.ascontiguousarray(inputs["H_O_edges"].reshape(NFRAMES, ROWS, D))
    On = inputs["O_nodes"].reshape(NFRAMES, O, D)
    Hn = inputs["H_nodes"].reshape(NFRAMES, H, D)
    Sc4 = inputs["S_node_C4"].reshape(NFRAMES, D)
    Sf = np.ascontiguousarray(inputs["final_S_node"].transpose(0, 2, 1)).reshape(NFRAMES, D)
    Hn_rm = Hn.reshape(NR, D)  # rows = (frame, h)

    bB_rz_full = inputs["gh_bih"] + inputs["gh_bhh"]
    bC_rz_full = inputs["gs_bih"] + inputs["gs_bhh"]

    shared = {
        "wcat": np.ascontiguousarray(
            np.concatenate([inputs["We"], inputs["Wl1"]], axis=0).T).astype(NB),
        "bl1t": np.ascontiguousarray(inputs["bl1"].reshape(8, 128).T).astype(np.float32),
        "bet": np.ascontiguousarray(inputs["be"].reshape(8, 128).T).astype(np.float32),
        "wnt": np.ascontiguousarray(inputs["Wn"].T).astype(NB),
        "wnb": inputs["bn"][None, :].astype(NB),
        "wl2": np.ascontiguousarray(inputs["Wl2"][0].reshape(8, 128).T).astype(NB),
        "htg": np.ascontiguousarray(Hn_rm.T).astype(NB),
        "pmat": np.ascontiguousarray(np.kron(np.eye(NFRAMES), np.ones((H, 1))) / H).astype(NB),
        "scsf": np.ascontiguousarray(np.concatenate([Sc4.T, Sf.T], axis=1)).astype(NB),
    }

    in_maps = []
    for c in range(NCORES):
        fr = slice(c * FPC, (c + 1) * FPC)
        Ec = E[fr]  # [16, 128, 2048]
        e0t = np.ascontiguousarray(
            Ec.reshape(NQ, 4, ROWS, D).transpose(0, 3, 1, 2).reshape(NQ, D, 512)).astype(NB)
        ot = np.ascontiguousarray(
            On[fr].reshape(FPC * O, D).T).astype(NB)
        hs = slice(c * HS, (c + 1) * HS)
        rows_rzn = np.r_[c * HS:(c + 1) * HS,
                         D + c * HS:D + (c + 1) * HS,
                         2 * D + c * HS:2 * D + (c + 1) * HS]
        rows_rz = rows_rzn[:2 * HS]
        m = dict(shared)
        m.update({
            "e0t": e0t,
            "ot": ot,
            "hrms": np.ascontiguousarray(Hn_rm[:, hs]).astype(np.float32),
            "wBi": np.ascontiguousarray((inputs["gh_wih"][rows_rzn] / float(O)).T).astype(NB),
            "wBh": np.ascontiguousarray(inputs["gh_whh"][rows_rzn].T).astype(NB),
            "bB_rz": bB_rz_full[rows_rz][None, :].astype(NB),
            "bB_in": inputs["gh_bih"][rows_rzn[2 * HS:]][None, :].astype(NB),
            "bB_hn": inputs["gh_bhh"][rows_rzn[2 * HS:]][None, :].astype(NB),
            "sc4s": np.ascontiguousarray(Sc4[:, hs]).astype(np.float32),
            "sfs": np.ascontiguousarray(Sf[:, hs]).astype(np.float32),
            "wCi": np.ascontiguousarray(inputs["gs_wih"][rows_rzn].T).astype(NB),
            "wCh": np.ascontiguousarray(inputs["gs_whh"][rows_rzn].T).astype(NB),
            "bC_rz": bC_rz_full[rows_rz][None, :].astype(NB),
            "bC_in": inputs["gs_bih"][rows_rzn[2 * HS:]][None, :].astype(NB),
            "bC_hn": inputs["gs_bhh"][rows_rzn[2 * HS:]][None, :].astype(NB),
        })
        in_maps.append(m)
    return in_maps


LAST_RESULT = None


def kernel(**inputs):
    global LAST_RESULT
    if "nc" not in _CACHE:
        _CACHE["nc"] = _build_nc()
    nc = _CACHE["nc"]
    in_maps = _prep_in_maps(inputs)
    trace = os.environ.get("KERNEL_TRACE", "0") == "1"
    res = bass_utils.run_bass_kernel_spmd(
        nc, in_maps, core_ids=list(range(NCORES)), trace=trace)
    LAST_RESULT = res
    out = np.empty((NFRAMES, D), np.float32)
    for c in range(NCORES):
        out[:, c * HS:(c + 1) * HS] = res.results[c]["outp"]
    return np.ascontiguousarray(out.reshape(B, F, D)).astype(np.float32)


if __name__ == "__main__":
    np.random.seed(0)
    ins = {
        "S_node_C4": np.random.randn(B, F, D).astype(np.float32),
        "final_S_node": np.random.randn(B, D, F).astype(np.float32),
        "H_nodes": np.random.randn(B, F, H, D).astype(np.float32),
        "O_nodes": np.random.randn(B, F, O, D).astype(np.float32),
        "H_O_edges": np.random.randn(B, F, H, O, D).astype(np.float32),
        "Wn": np.random.randn(D // 2, D).astype(np.float32) * 0.02,
        "bn": np.random.randn(D // 2).astype(np.float32) * 0.02,
        "We": np.random.randn(D // 2, D).astype(np.float32) * 0.02,
        "be": np.random.randn(D // 2).astype(np.float32) * 0.02,
        "Wl1": np.random.randn(D // 2, D).astype(np.float32) * 0.02,
        "bl1": np.random.randn(D // 2).astype(np.float32) * 0.02,
        "Wl2": np.random.randn(1, D // 2).astype(np.float32) * 0.02,
        "bl2": np.random.randn(1).astype(np.float32) * 0.02,
        "gh_wih": np.random.randn(3 * D, D).astype(np.float32) * 0.02,
        "gh_whh": np.random.randn(3 * D, D).astype(np.float32) * 0.02,
        "gh_bih": np.random.randn(3 * D).astype(np.float32) * 0.02,
        "gh_bhh": np.random.randn(3 * D).astype(np.float32) * 0.02,
        "gs_wih": np.random.randn(3 * D, D).astype(np.float32) * 0.02,
        "gs_whh": np.random.randn(3 * D, D).astype(np.float32) * 0.02,
        "gs_bih": np.random.randn(3 * D).astype(np.float32) * 0.02,
        "gs_bhh": np.random.randn(3 * D).astype(np.float32) * 0.02,
    }
    out = kernel(**ins)
    print("kernel ran, out shape", out.shape, out.dtype, float(np.abs(out).mean()))


# revision 26
# speedup vs baseline: 152.9560x; 1.4756x over previous
"""Trainium2 Bass kernel for nn_Graph_Enhance_model (GNN message passing).

Self-contained: hardcodes shapes B=4,F=32,H=8,O=16,D=2048, 8 cores.
Phase A (edge waves): data-parallel over the 128 (b,f) frames, 16/core.
Phases B/C (GRUs): tensor-parallel over the hidden dim (256 hidden
units per core, 768 of the 6144 gate rows), stitched with AllGathers.
"""

import os
import sys

for _p in ("/opt/trn_rl_repo", "/opt/pypackages"):
    if _p not in sys.path and os.path.isdir(_p):
        sys.path.append(_p)

import numpy as np
import ml_dtypes

import concourse.bass as bass
import concourse.bacc as bacc
import concourse.tile as tile
import concourse.mybir as mybir
from concourse import bass_utils
from concourse.masks import make_identity

BF16 = mybir.dt.bfloat16
F8 = mybir.dt.float8e4
F32 = mybir.dt.float32
DR = mybir.MatmulPerfMode.DoubleRow
AF = mybir.ActivationFunctionType
ALU = mybir.AluOpType
AX = mybir.AxisListType

NB = ml_dtypes.bfloat16
NF8 = ml_dtypes.float8_e4m3
KC2 = 8  # D-chunk pairs for fp8 DoubleRow

B, F, H, O, D = 4, 32, 8, 16, 2048
NFRAMES = B * F          # 128
NCORES = 8
FPC = NFRAMES // NCORES  # 16 frames per core (phase A)
ROWS = H * O             # 128 edge rows per frame
KC = D // 128            # 16 K-chunks
NQ = FPC // 4            # 4 quads of 4 frames
HS = D // NCORES         # 256 hidden units per core (phases B/C)
NR = NFRAMES * H         # 1024 human rows globally
RC = NR // 128           # 8 row-chunks of 128

_CACHE = {}


def _combine_e(nc, step, mt, q, pe, wb_e, bet_sb, um8, msum_b, pool):
    """UM = (msg_e_psum + be) * w ; step 1 also reduces over o into msum.

    Scales: step 0 pe = 64*x, bet = 64*be, wb_e = w/4 -> out = 16*UM (fp8).
            step 1 pe = 1024*x, bet = 1024*be, wb_e = w/1024 -> out = UM.
    """
    if step == 0:
        nc.vector.scalar_tensor_tensor(out=um8[:, mt // 2, mt % 2, :], in0=pe,
                                       scalar=bet_sb[:, mt:mt + 1], in1=wb_e,
                                       op0=ALU.add, op1=ALU.mult)
    else:
        tmp = pool.tile([128, 512], F32, tag="um2")
        nc.vector.scalar_tensor_tensor(out=tmp, in0=pe,
                                       scalar=bet_sb[:, mt:mt + 1], in1=wb_e,
                                       op0=ALU.add, op1=ALU.mult)
        with nc.allow_low_precision(reason="msum store bf16; o-reduce of 16 terms"):
            nc.vector.reduce_sum(msum_b[:, mt, q * 32:(q + 1) * 32],
                                 tmp.rearrange("p (f h o) -> p f h o", f=4, h=8),
                                 axis=AX.X)


def _build_nc():
    nc = bacc.Bacc("TRN2", target_bir_lowering=False, debug=False, num_devices=NCORES)

    dt_in = {}

    def din(name, shape, dt):
        dt_in[name] = nc.dram_tensor(name, shape, dt, kind="ExternalInput")
        return dt_in[name]

    # ---- phase A inputs (per-core frames) ----
    e0t = din("e0t", [NQ, D, 512], F8)
    ot = din("ot", [D, FPC * O], F8)
    wcat = din("wcat", [D, D], F8)        # 64*[We | Wl1]^T
    bl1td = din("bl1t", [128, 8], F32)    # 64*bl1
    betd64 = din("bet64", [128, 8], F32)  # 64*be
    betd1k = din("bet1k", [128, 8], F32)  # 1024*be
    wnt = din("wnt", [D, D // 2], F8)     # 64*Wn^T
    wnb = din("wnb", [1, D // 2], BF16)   # 64*bn
    wl2 = din("wl2", [128, 8], BF16)
    # ---- phase B inputs (all frames; per-core hidden slice) ----
    htg = din("htg", [D, NR], BF16)          # H_nodes^T, all 128 frames
    pmatd = din("pmat", [NR, NFRAMES], BF16)  # mean-over-H matrix /8
    hrms = din("hrms", [NR, HS], F32)        # H_nodes rows, hidden slice
    wBi_d = din("wBi", [D, 3 * HS], BF16)    # (gh_wih/16)[rzn slice]^T
    wBh_d = din("wBh", [D, 3 * HS], BF16)    # gh_whh[rzn slice]^T
    bB_rz_d = din("bB_rz", [1, 2 * HS], BF16)
    bB_in_d = din("bB_in", [1, HS], BF16)
    bB_hn_d = din("bB_hn", [1, HS], BF16)
    # ---- phase C inputs ----
    scsf_d = din("scsf", [D, 2 * NFRAMES], BF16)   # [S_C4^T | S_f^T]
    sc4s_d = din("sc4s", [NFRAMES, HS], F32)
    sfs_d = din("sfs", [NFRAMES, HS], F32)
    wCi_d = din("wCi", [D, 3 * HS], BF16)
    wCh_d = din("wCh", [D, 3 * HS], BF16)
    bC_rz_d = din("bC_rz", [1, 2 * HS], BF16)      # (bih+bhh)[rz]
    bC_in_d = din("bC_in", [1, HS], BF16)
    bC_hn_d = din("bC_hn", [1, HS], BF16)

    outp = nc.dram_tensor("outp", [NFRAMES, HS], F32, kind="ExternalOutput")

    from contextlib import ExitStack

    RG = [list(range(NCORES))]

    with tile.TileContext(nc) as tc, ExitStack() as ctx:
        glob = ctx.enter_context(tc.tile_pool(name="glob", bufs=1))
        dram = ctx.enter_context(tc.tile_pool(name="dram", bufs=1, space="DRAM"))

        ag1_ins = [dram.tile([D, 32], BF16, name=f"ag1i{q}") for q in range(NQ)]
        ag1_outs = [dram.tile([NCORES * D, 32], BF16, addr_space="Shared",
                              name=f"ag1o{q}") for q in range(NQ)]
        ag2_in = dram.tile([2 * 128, NFRAMES], BF16)
        ag2_out = dram.tile([D, NFRAMES], BF16, addr_space="Shared")
        ag3_in = dram.tile([2 * 128, NFRAMES], BF16)
        ag3_out = dram.tile([D, NFRAMES], BF16, addr_space="Shared")

        ones_b = glob.tile([1, 512], BF16)
        nc.vector.memset(ones_b, 1.0)
        cst_q = glob.tile([1, 128], BF16)     # 1/4: step-0 e-half w scale
        nc.vector.memset(cst_q, 0.25)
        cst_16 = glob.tile([1, 128], BF16)    # 16: step-0 n-half w scale
        nc.vector.memset(cst_16, 16.0)
        cst_1k = glob.tile([1, 128], BF16)    # 1/1024: step-1 e-half w scale
        nc.vector.memset(cst_1k, 1.0 / 1024.0)
        ident = glob.tile([128, 128], BF16)
        make_identity(nc, ident)

        wl2_sb = glob.tile([128, 8], BF16)
        nc.sync.dma_start(out=wl2_sb, in_=wl2.ap())
        bl1t_sb = glob.tile([128, 8], F32)
        nc.sync.dma_start(out=bl1t_sb, in_=bl1td.ap())
        bet64_sb = glob.tile([128, 8], F32)
        nc.sync.dma_start(out=bet64_sb, in_=betd64.ap())
        bet1k_sb = glob.tile([128, 8], F32)
        nc.sync.dma_start(out=bet1k_sb, in_=betd1k.ap())

        msgn_sb = glob.tile([128, 8, FPC * O], BF16)    # [1024, 256] transposed msg_n
        msum_b = glob.tile([128, KC, FPC * H], BF16)    # M_sum2^T (raw sum over o)

        with (
            tc.tile_pool(name="pwcat", bufs=1) as pwcat,
            tc.tile_pool(name="pa", bufs=2) as pa,
            tc.tile_pool(name="pa1", bufs=1) as pa1,
        ):
            # ---------------- Phase 0: msg_n^T = Wn @ O^T + bn (fp8, x64) ----------------
            wcat_sb = pwcat.tile([128, KC2, 2, D], F8)
            with (
                tc.tile_pool(name="p0", bufs=1) as p0,
                tc.tile_pool(name="p0ps", bufs=4, space="PSUM") as p0ps,
            ):
                wnb_sb = p0.tile([1, D // 2], BF16)
                nc.sync.dma_start(out=wnb_sb, in_=wnb.ap())
                ot_sb = p0.tile([128, KC2, 2, FPC * O], F8)
                nc.sync.dma_start(out=ot_sb, in_=ot.ap()
                                  .rearrange("(kc2 two p) n -> p kc2 two n", p=128, two=2))
                wn_sb = p0.tile([128, KC2, 2, D // 2], F8)
                nc.sync.dma_start(out=wn_sb, in_=wnt.ap()
                                  .rearrange("(kc2 two p) m -> p kc2 two m", p=128, two=2))
                # wcat load issued after phase-0 inputs so PE can start sooner
                nc.sync.dma_start(out=wcat_sb,
                                  in_=wcat.ap().rearrange("(kc2 two p) m -> p kc2 two m",
                                                          p=128, two=2))
                for mt in range(8):
                    pm = p0ps.tile([128, FPC * O], F32, tag="pm")
                    for kc2 in range(KC2):
                        nc.tensor.matmul(pm, lhsT=wn_sb[:, kc2, :, mt * 128:(mt + 1) * 128],
                                         rhs=ot_sb[:, kc2], start=(kc2 == 0), stop=False,
                                         perf_mode=DR)
                    nc.tensor.matmul(pm, lhsT=wnb_sb[0:1, mt * 128:(mt + 1) * 128],
                                     rhs=ones_b[0:1, 0:FPC * O], start=False, stop=True)
                    nc.scalar.activation(msgn_sb[:, mt, :], pm, AF.Copy, scale=1.0 / 64.0)

            # ---------------- Phase A: 2 propagation steps over edges ----------------
            with tc.tile_pool(name="paps", bufs=4, space="PSUM") as paps, \
                 tc.tile_pool(name="papss", bufs=2, space="PSUM") as papss:
                for q in range(NQ):
                    xq = pa.tile([128, KC2, 2, 512], F8, tag="xq")
                    nc.sync.dma_start(out=xq, in_=e0t.ap()[q]
                                      .rearrange("(kc2 two p) n -> p kc2 two n", p=128, two=2))
                    um1t = pa1.tile([128, KC2, 2, 512], F8, tag="um1t")
                    for step in range(2):
                        rhs = xq if step == 0 else um1t
                        rscale = 1.0 if step == 0 else 1.0 / 16.0
                        bet_sb = bet64_sb if step == 0 else bet1k_sb
                        cst_e = cst_q if step == 0 else cst_1k
                        cst_n = cst_16 if step == 0 else ones_b
                        # --- a-wave: relu(X @ Wl1^T + bl1), transposed ---
                        relu_sb = pa1.tile([128, 8, 512], BF16, tag="relu")
                        for mt in range(8, 16):
                            pw_a = paps.tile([128, 512], F32, tag="wave")
                            for kc2 in range(KC2):
                                nc.tensor.matmul(pw_a,
                                                 lhsT=wcat_sb[:, kc2, :, mt * 128:(mt + 1) * 128],
                                                 rhs=rhs[:, kc2], start=(kc2 == 0),
                                                 stop=(kc2 == KC2 - 1), perf_mode=DR)
                            nc.scalar.activation(relu_sb[:, mt - 8, :], pw_a, AF.Relu,
                                                 bias=bl1t_sb[:, mt - 8:mt - 7], scale=rscale)
                        # --- logits + softmax over o (groups of 16) ---
                        pl = papss.tile([1, 512], F32, tag="pl")
                        for kc2 in range(8):
                            nc.tensor.matmul(pl, lhsT=wl2_sb[:, kc2:kc2 + 1],
                                             rhs=relu_sb[:, kc2, :], start=(kc2 == 0), stop=(kc2 == 7))
                        pl3 = pl.rearrange("o (g i) -> o g i", i=16)
                        mx = pa1.tile([1, 32], F32, tag="mx")
                        nc.vector.reduce_max(mx, pl3, axis=AX.X)
                        sub = pa1.tile([1, 512], F32, tag="sub")
                        nc.vector.tensor_tensor(sub.rearrange("o (g i) -> o g i", i=16), pl3,
                                                mx.broadcast_to((1, 32, 16)), op=ALU.subtract)
                        nc.scalar.activation(sub, sub, AF.Exp)
                        ex3 = sub.rearrange("o (g i) -> o g i", i=16)
                        sm = pa1.tile([1, 32], F32, tag="sm")
                        nc.vector.reduce_sum(sm, ex3, axis=AX.X)
                        rs = pa1.tile([1, 32], F32, tag="rs")
                        nc.vector.reciprocal(rs, sm)
                        w_sb = pa1.tile([1, 512], BF16, tag="w")
                        nc.vector.tensor_tensor(w_sb.rearrange("o (g i) -> o g i", i=16), ex3,
                                                rs.broadcast_to((1, 32, 16)), op=ALU.mult)
                        # --- msg_e wave; w-broadcast MMs emitted after 2 groups ---
                        e_ps = []
                        wb_e = pa1.tile([128, 512], F32, tag="wbe")
                        wb_n = pa1.tile([128, 512], F32, tag="wbn")
                        for mt in range(8):
                            pe = paps.tile([128, 512], F32, tag="wave")
                            for kc2 in range(KC2):
                                nc.tensor.matmul(pe,
                                                 lhsT=wcat_sb[:, kc2, :, mt * 128:(mt + 1) * 128],
                                                 rhs=rhs[:, kc2], start=(kc2 == 0),
                                                 stop=(kc2 == KC2 - 1), perf_mode=DR)
                            e_ps.append(pe)
                            if mt == 1:
                                # broadcast scaled w along partitions via K=1 matmuls
                                # (PE waits here on softmax, hidden under 2 MM groups)
                                pw_b = papss.tile([128, 512], F32, tag="pw")
                                nc.tensor.matmul(pw_b, lhsT=cst_e[0:1, 0:128], rhs=w_sb,
                                                 start=True, stop=True)
                                nc.scalar.copy(wb_e, pw_b)
                                pw_c = papss.tile([128, 512], F32, tag="pw")
                                nc.tensor.matmul(pw_c, lhsT=cst_n[0:1, 0:128], rhs=w_sb,
                                                 start=True, stop=True)
                                nc.scalar.copy(wb_n, pw_c)
                            if mt >= 1:
                                for cmt in ([0, 1] if mt == 1 else [mt]):
                                    _combine_e(nc, step, cmt, q, e_ps[cmt], wb_e, bet_sb,
                                               um1t, msum_b, pa1)
                        wb4 = wb_n.rearrange("p (f h o) -> p f h o", f=4, h=8)
                        # msg_n half (tiles 8..16): broadcast over h
                        for j in range(8):
                            mt = 8 + j
                            base = msgn_sb[:, j, q * 64:(q + 1) * 64]
                            mn_bc = bass.AP(tensor=base.tensor, offset=base.offset,
                                            ap=[list(base.ap[0]), [16, 4], [0, 8], [1, 16]])
                            if step == 0:
                                nc.vector.tensor_tensor(
                                    um1t[:, mt // 2, mt % 2, :]
                                    .rearrange("p (f h o) -> p f h o", f=4, h=8),
                                    mn_bc, wb4, op=ALU.mult)
                            else:
                                tmp = pa1.tile([128, 512], F32, tag="um2")
                                nc.vector.tensor_tensor(
                                    tmp.rearrange("p (f h o) -> p f h o", f=4, h=8),
                                    mn_bc, wb4, op=ALU.mult)
                                with nc.allow_low_precision(
                                        reason="msum store bf16; o-reduce of 16 terms"):
                                    nc.vector.reduce_sum(
                                        msum_b[:, mt, q * 32:(q + 1) * 32],
                                        tmp.rearrange("p (f h o) -> p f h o", f=4, h=8),
                                        axis=AX.X)
                    # ---- AG1 chunk q: gather this quad's msum cols from all cores ----
                    nc.sync.dma_start(out=ag1_ins[q].rearrange("(kc p) n -> p kc n", p=128),
                                      in_=msum_b[:, :, q * 32:(q + 1) * 32])
                    nc.gpsimd.collective_compute(
                        "AllGather", ALU.bypass, replica_groups=RG,
                        ins=[ag1_ins[q].opt()], outs=[ag1_outs[q].opt()])

        # ---------------- Phase B: human GRU, hidden-slice parallel ----------------
        with (
            tc.tile_pool(name="pbw", bufs=1) as pbw,
            tc.tile_pool(name="pbs", bufs=2) as pbs,
            tc.tile_pool(name="pb1", bufs=1) as pb1,
            tc.tile_pool(name="pbps", bufs=2, space="PSUM") as pbps,
            tc.tile_pool(name="pbps2", bufs=2, space="PSUM") as pbps2,
            tc.tile_pool(name="pcw", bufs=1) as pcw,
        ):
            wBh_sb = pbw.tile([128, KC, 3 * HS], BF16)
            nc.sync.dma_start(out=wBh_sb, in_=wBh_d.ap().rearrange("(kc p) m -> p kc m", p=128))
            wBi_sb = pbw.tile([128, KC, 3 * HS], BF16)
            nc.sync.dma_start(out=wBi_sb, in_=wBi_d.ap().rearrange("(kc p) m -> p kc m", p=128))
            bB_rz = pbw.tile([1, 2 * HS], BF16)
            nc.sync.dma_start(out=bB_rz, in_=bB_rz_d.ap())
            bB_in = pbw.tile([1, HS], BF16)
            nc.sync.dma_start(out=bB_in, in_=bB_in_d.ap())
            bB_hn = pbw.tile([1, HS], BF16)
            nc.sync.dma_start(out=bB_hn, in_=bB_hn_d.ap())
            pmat_sb = pbw.tile([128, RC, NFRAMES], BF16)
            nc.sync.dma_start(out=pmat_sb, in_=pmatd.ap().rearrange("(rc p) f -> p rc f", p=128))
            hrm_sb = pbw.tile([128, RC, HS], F32)
            nc.sync.dma_start(out=hrm_sb, in_=hrms.ap().rearrange("(rc p) m -> p rc m", p=128))
            hum_sb = pbw.tile([128, RC, HS], BF16)

            for rc in range(RC):
                ht_t = pbs.tile([128, KC, 128], BF16, tag="ht")
                nc.sync.dma_start(out=ht_t, in_=htg.ap()[:, rc * 128:(rc + 1) * 128]
                                  .rearrange("(kc p) n -> p kc n", p=128))
                ms_t = pbs.tile([128, KC, 128], BF16, tag="ms")
                for q in range(NQ):
                    nc.sync.dma_start(out=ms_t[:, :, q * 32:(q + 1) * 32],
                                      in_=ag1_outs[q][rc * D:(rc + 1) * D, :]
                                      .rearrange("(kc p) n -> p kc n", p=128))
                p_rz = pbps.tile([128, 2 * HS], F32, tag="rz")
                for kc in range(KC):
                    nc.tensor.matmul(p_rz, lhsT=ht_t[:, kc, :], rhs=wBh_sb[:, kc, 0:2 * HS],
                                     start=(kc == 0), stop=False)
                for kc in range(KC):
                    nc.tensor.matmul(p_rz, lhsT=ms_t[:, kc, :], rhs=wBi_sb[:, kc, 0:2 * HS],
                                     start=False, stop=False)
                nc.tensor.matmul(p_rz, lhsT=ones_b[0:1, 0:128], rhs=bB_rz[0:1, :],
                                 start=False, stop=True)
                p_hn = pbps.tile([128, HS], F32, tag="hn")
                for kc in range(KC):
                    nc.tensor.matmul(p_hn, lhsT=ht_t[:, kc, :], rhs=wBh_sb[:, kc, 2 * HS:3 * HS],
                                     start=(kc == 0), stop=False)
                nc.tensor.matmul(p_hn, lhsT=ones_b[0:1, 0:128], rhs=bB_hn[0:1, :],
                                 start=False, stop=True)
                p_in = pbps.tile([128, HS], F32, tag="in")
                for kc in range(KC):
                    nc.tensor.matmul(p_in, lhsT=ms_t[:, kc, :], rhs=wBi_sb[:, kc, 2 * HS:3 * HS],
                                     start=(kc == 0), stop=False)
                nc.tensor.matmul(p_in, lhsT=ones_b[0:1, 0:128], rhs=bB_in[0:1, :],
                                 start=False, stop=True)
                r_sb = pb1.tile([128, HS], F32, tag="r")
                nc.scalar.activation(r_sb, p_rz[:, 0:HS], AF.Sigmoid)
                z_sb = pb1.tile([128, HS], F32, tag="z")
                nc.scalar.activation(z_sb, p_rz[:, HS:2 * HS], AF.Sigmoid)
                t1 = pb1.tile([128, HS], F32, tag="t1")
                nc.vector.tensor_tensor(t1, r_sb, p_hn, op=ALU.mult)
                t2 = pb1.tile([128, HS], F32, tag="t2")
                nc.vector.tensor_tensor(t2, t1, p_in, op=ALU.add)
                n_sb = pb1.tile([128, HS], F32, tag="n")
                nc.scalar.activation(n_sb, t2, AF.Tanh)
                t3 = pb1.tile([128, HS], F32, tag="t3")
                nc.vector.tensor_tensor(t3, hrm_sb[:, rc, :], n_sb, op=ALU.subtract)
                t4 = pb1.tile([128, HS], F32, tag="t4")
                nc.vector.tensor_tensor(t4, z_sb, t3, op=ALU.mult)
                nc.vector.tensor_tensor(hum_sb[:, rc, :], n_sb, t4, op=ALU.add)

            # All_human^T slice: ah[mc] = hum[:, mc-chunk].T @ pmat   [256, 128]
            ahT_sb = pb1.tile([128, 2, NFRAMES], BF16, tag="ahT")
            for mc in range(2):
                pah = pbps2.tile([128, NFRAMES], F32, tag="pah")
                for rc in range(RC):
                    nc.tensor.matmul(pah, lhsT=hum_sb[:, rc, mc * 128:(mc + 1) * 128],
                                     rhs=pmat_sb[:, rc, :], start=(rc == 0), stop=(rc == RC - 1))
                nc.scalar.copy(ahT_sb[:, mc, :], pah)
            nc.sync.dma_start(out=ag2_in.rearrange("(mc p) f -> p mc f", p=128),
                              in_=ahT_sb)
            nc.gpsimd.collective_compute(
                "AllGather", ALU.bypass, replica_groups=RG,
                ins=[ag2_in.opt()], outs=[ag2_out.opt()])

            # -------- Phase C prefetched weights (loaded during B) --------
            wCh_sb = pcw.tile([128, KC, 3 * HS], BF16)
            nc.sync.dma_start(out=wCh_sb, in_=wCh_d.ap().rearrange("(kc p) m -> p kc m", p=128))
            wCi_sb = pcw.tile([128, KC, 3 * HS], BF16)
            nc.sync.dma_start(out=wCi_sb, in_=wCi_d.ap().rearrange("(kc p) m -> p kc m", p=128))
            scsf_sb = pcw.tile([128, KC, 2 * NFRAMES], BF16)
            nc.sync.dma_start(out=scsf_sb, in_=scsf_d.ap().rearrange("(kc p) n -> p kc n", p=128))
            bC_rz = pcw.tile([1, 2 * HS], BF16)
            nc.sync.dma_start(out=bC_rz, in_=bC_rz_d.ap())
            bC_in = pcw.tile([1, HS], BF16)
            nc.sync.dma_start(out=bC_in, in_=bC_in_d.ap())
            bC_hn = pcw.tile([1, HS], BF16)
            nc.sync.dma_start(out=bC_hn, in_=bC_hn_d.ap())
            sc4s_sb = pcw.tile([NFRAMES, HS], F32)
            nc.sync.dma_start(out=sc4s_sb, in_=sc4s_d.ap())
            sfs_sb = pcw.tile([NFRAMES, HS], F32)
            nc.sync.dma_start(out=sfs_sb, in_=sfs_d.ap())

        # ---------------- Phase C: two S-node GRUs, hidden-slice parallel ----------------
        with (
            tc.tile_pool(name="pc1", bufs=1) as pc1,
            tc.tile_pool(name="pcps", bufs=2, space="PSUM") as pcps,
            tc.tile_pool(name="pctps", bufs=2, space="PSUM") as pctps,
        ):
            ah_sb = pc1.tile([128, KC, NFRAMES], BF16)
            nc.sync.dma_start(out=ah_sb, in_=ag2_out
                              .rearrange("(kc p) f -> p kc f", p=128))

            # gh2 (vs S_f), biases folded in; stored for step 2
            g2h_rz = pc1.tile([NFRAMES, 2 * HS], BF16)
            p2h = pcps.tile([NFRAMES, 2 * HS], F32, tag="rz")
            for kc in range(KC):
                nc.tensor.matmul(p2h, lhsT=scsf_sb[:, kc, NFRAMES:2 * NFRAMES],
                                 rhs=wCh_sb[:, kc, 0:2 * HS], start=(kc == 0), stop=False)
            nc.tensor.matmul(p2h, lhsT=ones_b[0:1, 0:NFRAMES], rhs=bC_rz[0:1, :],
                             start=False, stop=True)
            nc.scalar.copy(g2h_rz, p2h)
            g2h_hn = pc1.tile([NFRAMES, HS], BF16)
            p2hn = pcps.tile([NFRAMES, HS], F32, tag="hn")
            for kc in range(KC):
                nc.tensor.matmul(p2hn, lhsT=scsf_sb[:, kc, NFRAMES:2 * NFRAMES],
                                 rhs=wCh_sb[:, kc, 2 * HS:3 * HS], start=(kc == 0), stop=False)
            nc.tensor.matmul(p2hn, lhsT=ones_b[0:1, 0:NFRAMES], rhs=bC_hn[0:1, :],
                             start=False, stop=True)
            nc.scalar.copy(g2h_hn, p2hn)

            # step-1 gates: gh1(S_C4) first (AG2-independent), then gi1(AH)
            p1_hn = pcps.tile([NFRAMES, HS], F32, tag="hn")
            for kc in range(KC):
                nc.tensor.matmul(p1_hn, lhsT=scsf_sb[:, kc, 0:NFRAMES],
                                 rhs=wCh_sb[:, kc, 2 * HS:3 * HS], start=(kc == 0), stop=False)
            nc.tensor.matmul(p1_hn, lhsT=ones_b[0:1, 0:NFRAMES], rhs=bC_hn[0:1, :],
                             start=False, stop=True)
            p1_rz = pcps.tile([NFRAMES, 2 * HS], F32, tag="rz")
            for kc in range(KC):
                nc.tensor.matmul(p1_rz, lhsT=scsf_sb[:, kc, 0:NFRAMES],
                                 rhs=wCh_sb[:, kc, 0:2 * HS], start=(kc == 0), stop=False)
            for kc in range(KC):
                nc.tensor.matmul(p1_rz, lhsT=ah_sb[:, kc, :], rhs=wCi_sb[:, kc, 0:2 * HS],
                                 start=False, stop=False)
            nc.tensor.matmul(p1_rz, lhsT=ones_b[0:1, 0:NFRAMES], rhs=bC_rz[0:1, :],
                             start=False, stop=True)
            p1_in = pcps.tile([NFRAMES, HS], F32, tag="in")
            for kc in range(KC):
                nc.tensor.matmul(p1_in, lhsT=ah_sb[:, kc, :], rhs=wCi_sb[:, kc, 2 * HS:3 * HS],
                                 start=(kc == 0), stop=False)
            nc.tensor.matmul(p1_in, lhsT=ones_b[0:1, 0:NFRAMES], rhs=bC_in[0:1, :],
                             start=False, stop=True)

            # step-1 elementwise -> s1 slice
            z1 = pc1.tile([NFRAMES, HS], F32, tag="z1")
            nc.scalar.activation(z1, p1_rz[:, HS:2 * HS], AF.Sigmoid)
            r1 = pc1.tile([NFRAMES, HS], F32, tag="r1")
            nc.scalar.activation(r1, p1_rz[:, 0:HS], AF.Sigmoid)
            u1 = pc1.tile([NFRAMES, HS], F32, tag="u1")
            nc.vector.tensor_tensor(u1, r1, p1_hn, op=ALU.mult)
            u2 = pc1.tile([NFRAMES, HS], F32, tag="u2")
            nc.vector.tensor_tensor(u2, u1, p1_in, op=ALU.add)
            n1 = pc1.tile([NFRAMES, HS], F32, tag="n1")
            nc.scalar.activation(n1, u2, AF.Tanh)
            u3 = pc1.tile([NFRAMES, HS], F32, tag="u3")
            nc.vector.tensor_tensor(u3, sc4s_sb, n1, op=ALU.subtract)
            u4 = pc1.tile([NFRAMES, HS], F32, tag="u4")
            nc.vector.tensor_tensor(u4, z1, u3, op=ALU.mult)
            s1_sb = pc1.tile([NFRAMES, HS], BF16, tag="s1")
            nc.vector.tensor_tensor(s1_sb, n1, u4, op=ALU.add)

            # transpose s1 slice -> [256, 128] and AllGather full s1^T
            s1t_sb = pc1.tile([128, 2, NFRAMES], BF16, tag="s1t")
            for mc in range(2):
                ptp = pctps.tile([128, NFRAMES], BF16, tag="tp")
                nc.tensor.transpose(ptp, s1_sb[:, mc * 128:(mc + 1) * 128], ident)
                nc.scalar.copy(s1t_sb[:, mc, :], ptp)
            nc.sync.dma_start(out=ag3_in.rearrange("(mc p) f -> p mc f", p=128),
                              in_=s1t_sb)
            nc.gpsimd.collective_compute(
                "AllGather", ALU.bypass, replica_groups=RG,
                ins=[ag3_in.opt()], outs=[ag3_out.opt()])
            s1t_g = pc1.tile([128, KC, NFRAMES], BF16)
            nc.sync.dma_start(out=s1t_g, in_=ag3_out
                              .rearrange("(kc p) f -> p kc f", p=128))

            # step-2 gates: gi2(s1); gh2 already in SBUF
            p2_rz = pcps.tile([NFRAMES, 2 * HS], F32, tag="rz")
            for kc in range(KC):
                nc.tensor.matmul(p2_rz, lhsT=s1t_g[:, kc, :], rhs=wCi_sb[:, kc, 0:2 * HS],
                                 start=(kc == 0), stop=(kc == KC - 1))
            p2_n = pcps.tile([NFRAMES, HS], F32, tag="in")
            for kc in range(KC):
                nc.tensor.matmul(p2_n, lhsT=s1t_g[:, kc, :], rhs=wCi_sb[:, kc, 2 * HS:3 * HS],
                                 start=(kc == 0), stop=False)
            nc.tensor.matmul(p2_n, lhsT=ones_b[0:1, 0:NFRAMES], rhs=bC_in[0:1, :],
                             start=False, stop=True)

            # step-2 elementwise -> out slice
            grz = pc1.tile([NFRAMES, 2 * HS], F32, tag="grz")
            nc.vector.tensor_tensor(grz, p2_rz, g2h_rz, op=ALU.add)
            z2 = pc1.tile([NFRAMES, HS], F32, tag="z2")
            nc.scalar.activation(z2, grz[:, HS:2 * HS], AF.Sigmoid)
            r2 = pc1.tile([NFRAMES, HS], F32, tag="r2")
            nc.scalar.activation(r2, grz[:, 0:HS], AF.Sigmoid)
            v1 = pc1.tile([NFRAMES, HS], F32, tag="v1")
            nc.vector.tensor_tensor(v1, r2, g2h_hn, op=ALU.mult)
            v2 = pc1.tile([NFRAMES, HS], F32, tag="v2")
            nc.vector.tensor_tensor(v2, v1, p2_n, op=ALU.add)
            n2 = pc1.tile([NFRAMES, HS], F32, tag="n2")
            nc.scalar.activation(n2, v2, AF.Tanh)
            v3 = pc1.tile([NFRAMES, HS], F32, tag="v3")
            nc.vector.tensor_tensor(v3, sfs_sb, n2, op=ALU.subtract)
            v4 = pc1.tile([NFRAMES, HS], F32, tag="v4")
            nc.vector.tensor_tensor(v4, z2, v3, op=ALU.mult)
            out_sb = pc1.tile([NFRAMES, HS], F32, tag="out")
            nc.vector.tensor_tensor(out_sb, n2, v4, op=ALU.add)
            nc.sync.dma_start(out=outp.ap(), in_=out_sb)

    nc.compile()
    return nc


def _prep_in_maps(inputs):
    E = np.ascontiguousarray(inputs["H_O_edges"].reshape(NFRAMES, ROWS, D))
    On = inputs["O_nodes"].reshape(NFRAMES, O, D)
    Hn = inputs["H_nodes"].reshape(NFRAMES, H, D)
    Sc4 = inputs["S_node_C4"].reshape(NFRAMES, D)
    Sf = np.ascontiguousarray(inputs["final_S_node"].transpose(0, 2, 1)).reshape(NFRAMES, D)
    Hn_rm = Hn.reshape(NR, D)  # rows = (frame, h)

    bB_rz_full = inputs["gh_bih"] + inputs["gh_bhh"]
    bC_rz_full = inputs["gs_bih"] + inputs["gs_bhh"]

    shared = {
        "wcat": np.ascontiguousarray(
            64.0 * np.concatenate([inputs["We"], inputs["Wl1"]], axis=0).T).astype(NF8),
        "bl1t": np.ascontiguousarray(64.0 * inputs["bl1"].reshape(8, 128).T).astype(np.float32),
        "bet64": np.ascontiguousarray(64.0 * inputs["be"].reshape(8, 128).T).astype(np.float32),
        "bet1k": np.ascontiguousarray(1024.0 * inputs["be"].reshape(8, 128).T).astype(np.float32),
        "wnt": np.ascontiguousarray(64.0 * inputs["Wn"].T).astype(NF8),
        "wnb": 64.0 * inputs["bn"][None, :].astype(NB),
        "wl2": np.ascontiguousarray(inputs["Wl2"][0].reshape(8, 128).T / 64.0).astype(NB),
        "htg": np.ascontiguousarray(Hn_rm.T).astype(NB),
        "pmat": np.ascontiguousarray(np.kron(np.eye(NFRAMES), np.ones((H, 1))) / H).astype(NB),
        "scsf": np.ascontiguousarray(np.concatenate([Sc4.T, Sf.T], axis=1)).astype(NB),
    }

    in_maps = []
    for c in range(NCORES):
        fr = slice(c * FPC, (c + 1) * FPC)
        Ec = E[fr]  # [16, 128, 2048]
        e0t = np.ascontiguousarray(
            Ec.reshape(NQ, 4, ROWS, D).transpose(0, 3, 1, 2).reshape(NQ, D, 512)).astype(NF8)
        ot = np.ascontiguousarray(
            On[fr].reshape(FPC * O, D).T).astype(NF8)
        hs = slice(c * HS, (c + 1) * HS)
        rows_rzn = np.r_[c * HS:(c + 1) * HS,
                         D + c * HS:D + (c + 1) * HS,
                         2 * D + c * HS:2 * D + (c + 1) * HS]
        rows_rz = rows_rzn[:2 * HS]
        m = dict(shared)
        m.update({
            "e0t": e0t,
            "ot": ot,
            "hrms": np.ascontiguousarray(Hn_rm[:, hs]).astype(np.float32),
            "wBi": np.ascontiguousarray((inputs["gh_wih"][rows_rzn] / float(O)).T).astype(NB),
            "wBh": np.ascontiguousarray(inputs["gh_whh"][rows_rzn].T).astype(NB),
            "bB_rz": bB_rz_full[rows_rz][None, :].astype(NB),
            "bB_in": inputs["gh_bih"][rows_rzn[2 * HS:]][None, :].astype(NB),
            "bB_hn": inputs["gh_bhh"][rows_rzn[2 * HS:]][None, :].astype(NB),
            "sc4s": np.ascontiguousarray(Sc4[:, hs]).astype(np.float32),
            "sfs": np.ascontiguousarray(Sf[:, hs]).astype(np.float32),
            "wCi": np.ascontiguousarray(inputs["gs_wih"][rows_rzn].T).astype(NB),
            "wCh": np.ascontiguousarray(inputs["gs_whh"][rows_rzn].T).astype(NB),
            "bC_rz": bC_rz_full[rows_rz][None, :].astype(NB),
            "bC_in": inputs["gs_bih"][rows_rzn[2 * HS:]][None, :].astype(NB),
            "bC_hn": inputs["gs_bhh"][rows_rzn[2 * HS:]][None, :].astype(NB),
        })
        in_maps.append(m)
    return in_maps


LAST_RESULT = None


def kernel(**inputs):
    global LAST_RESULT
    if "nc" not in _CACHE:
        _CACHE["nc"] = _build_nc()
    nc = _CACHE["nc"]
    in_maps = _prep_in_maps(inputs)
    trace = os.environ.get("KERNEL_TRACE", "0") == "1"
    res = bass_utils.run_bass_kernel_spmd(
        nc, in_maps, core_ids=list(range(NCORES)), trace=trace)
    LAST_RESULT = res
    out = np.empty((NFRAMES, D), np.float32)
    for c in range(NCORES):
        out[:, c * HS:(c + 1) * HS] = res.results[c]["outp"]
    return np.ascontiguousarray(out.reshape(B, F, D)).astype(np.float32)


if __name__ == "__main__":
    np.random.seed(0)
    ins = {
        "S_node_C4": np.random.randn(B, F, D).astype(np.float32),
        "final_S_node": np.random.randn(B, D, F).astype(np.float32),
        "H_nodes": np.random.randn(B, F, H, D).astype(np.float32),
        "O_nodes": np.random.randn(B, F, O, D).astype(np.float32),
        "H_O_edges": np.random.randn(B, F, H, O, D).astype(np.float32),
        "Wn": np.random.randn(D // 2, D).astype(np.float32) * 0.02,
        "bn": np.random.randn(D // 2).astype(np.float32) * 0.02,
        "We": np.random.randn(D // 2, D).astype(np.float32) * 0.02,
        "be": np.random.randn(D // 2).astype(np.float32) * 0.02,
        "Wl1": np.random.randn(D // 2, D).astype(np.float32) * 0.02,
        "bl1": np.random.randn(D // 2).astype(np.float32) * 0.02,
        "Wl2": np.random.randn(1, D // 2).astype(np.float32) * 0.02,
        "bl2": np.random.randn(1).astype(np.float32) * 0.02,
        "gh_wih": np.random.randn(3 * D, D).astype(np.float32) * 0.02,
        "gh_whh": np.random.randn(3 * D, D).astype(np.float32) * 0.02,
        "gh_bih": np.random.randn(3 * D).astype(np.float32) * 0.02,
        "gh_bhh": np.random.randn(3 * D).astype(np.float32) * 0.02,
        "gs_wih": np.random.randn(3 * D, D).astype(np.float32) * 0.02,
        "gs_whh": np.random.randn(3 * D, D).astype(np.float32) * 0.02,
        "gs_bih": np.random.randn(3 * D).astype(np.float32) * 0.02,
        "gs_bhh": np.random.randn(3 * D).astype(np.float32) * 0.02,
    }
    out = kernel(**ins)
    print("kernel ran, out shape", out.shape, out.dtype, float(np.abs(out).mean()))


# revision 39
# speedup vs baseline: 161.2847x; 1.0545x over previous
"""Trainium2 Bass kernel for nn_Graph_Enhance_model (GNN message passing).

Self-contained: hardcodes shapes B=4,F=32,H=8,O=16,D=2048, 8 cores.
Phase A (edge waves): data-parallel over the 128 (b,f) frames, 16/core.
Phases B/C (GRUs): tensor-parallel over the hidden dim (256 hidden
units per core, 768 of the 6144 gate rows), stitched with AllGathers.
"""

import os
import sys

for _p in ("/opt/trn_rl_repo", "/opt/pypackages"):
    if _p not in sys.path and os.path.isdir(_p):
        sys.path.append(_p)

import numpy as np
import ml_dtypes

import concourse.bass as bass
import concourse.bacc as bacc
import concourse.tile as tile
import concourse.mybir as mybir
from concourse import bass_utils
from concourse.masks import make_identity

BF16 = mybir.dt.bfloat16
F8 = mybir.dt.float8e4
F32 = mybir.dt.float32
DR = mybir.MatmulPerfMode.DoubleRow
AF = mybir.ActivationFunctionType
ALU = mybir.AluOpType
AX = mybir.AxisListType

NB = ml_dtypes.bfloat16
NF8 = ml_dtypes.float8_e4m3
KC2 = 8  # D-chunk pairs for fp8 DoubleRow

B, F, H, O, D = 4, 32, 8, 16, 2048
NFRAMES = B * F          # 128
NCORES = 8
FPC = NFRAMES // NCORES  # 16 frames per core (phase A)
ROWS = H * O             # 128 edge rows per frame
KC = D // 128            # 16 K-chunks
NQ = FPC // 4            # 4 quads of 4 frames
HS = D // NCORES         # 256 hidden units per core (phases B/C)
NR = NFRAMES * H         # 1024 human rows globally
RC = NR // 128           # 8 row-chunks of 128

_CACHE = {}


def _combine_e(nc, step, mt, q, pe, wb_e, bet_sb, um8, msum_b, pool):
    """UM = (msg_e_psum + be) * w ; step 1 also reduces over o into msum.

    Scales: step 0 pe = 64*x, bet = 64*be, wb_e = w/4 -> out = 16*UM (fp8).
            step 1 pe = 1024*x, bet = 1024*be, wb_e = w/1024 -> out = UM.
    """
    if step == 0:
        nc.vector.scalar_tensor_tensor(out=um8[:, mt // 2, mt % 2, :], in0=pe,
                                       scalar=bet_sb[:, mt:mt + 1], in1=wb_e,
                                       op0=ALU.add, op1=ALU.mult)
    else:
        tmp = pool.tile([128, 512], F32, tag="um2")
        nc.vector.scalar_tensor_tensor(out=tmp, in0=pe,
                                       scalar=bet_sb[:, mt:mt + 1], in1=wb_e,
                                       op0=ALU.add, op1=ALU.mult)
        with nc.allow_low_precision(reason="msum store bf16; o-reduce of 16 terms"):
            nc.vector.reduce_sum(msum_b[:, mt, q * 32:(q + 1) * 32],
                                 tmp.rearrange("p (f h o) -> p f h o", f=4, h=8),
                                 axis=AX.X)


def _build_nc():
    nc = bacc.Bacc("TRN2", target_bir_lowering=False, debug=False, num_devices=NCORES)

    dt_in = {}

    def din(name, shape, dt):
        dt_in[name] = nc.dram_tensor(name, shape, dt, kind="ExternalInput")
        return dt_in[name]

    # ---- phase A inputs (per-core frames) ----
    e0t = din("e0t", [NQ, D, 512], F8)
    ot = din("ot", [D, FPC * O], F8)
    wcat = din("wcat", [D, D], F8)        # 64*[We | Wl1]^T
    bl1td = din("bl1t", [128, 8], F32)    # 64*bl1
    betd64 = din("bet64", [128, 8], F32)  # 64*be
    betd1k = din("bet1k", [128, 8], F32)  # 1024*be
    wnt = din("wnt", [D, D // 2], F8)     # 64*Wn^T
    wnb = din("wnb", [1, D // 2], BF16)   # 64*bn
    wl2 = din("wl2", [128, 8], BF16)
    # ---- phase B inputs (all frames; per-core hidden slice); fp8, weights x64 ----
    htg = din("htg", [D, NR], F8)            # H_nodes^T, all 128 frames
    pmatd = din("pmat", [NR, NFRAMES], BF16)  # mean-over-H matrix /8
    hrms = din("hrms", [NR, HS], F32)        # H_nodes rows, hidden slice
    wBi_d = din("wBi", [D, 3 * HS], F8)      # 64*(gh_wih/16)[rzn slice]^T
    wBh_d = din("wBh", [D, 3 * HS], F8)      # 64*gh_whh[rzn slice]^T
    bB_rz_d = din("bB_rz", [1, 2 * HS], BF16)  # 64*(bih+bhh)[rz]
    bB_in_d = din("bB_in", [1, HS], BF16)      # 64*bih[n]
    bB_hn_d = din("bB_hn", [1, HS], BF16)      # 64*bhh[n]
    # ---- phase C inputs ----
    scsf_d = din("scsf", [D, 2 * NFRAMES], BF16)   # [S_C4^T | S_f^T]
    sc4s_d = din("sc4s", [NFRAMES, HS], F32)
    sfs_d = din("sfs", [NFRAMES, HS], F32)
    wCi_d = din("wCi", [D, 3 * HS], BF16)
    wCh_d = din("wCh", [D, 3 * HS], BF16)
    bC_rz_d = din("bC_rz", [1, 2 * HS], BF16)      # (bih+bhh)[rz]
    bC_in_d = din("bC_in", [1, HS], BF16)
    bC_hn_d = din("bC_hn", [1, HS], BF16)

    outp = nc.dram_tensor("outp", [NFRAMES, HS], F32, kind="ExternalOutput")

    from contextlib import ExitStack

    RG = [list(range(NCORES))]

    with tile.TileContext(nc) as tc, ExitStack() as ctx:
        glob = ctx.enter_context(tc.tile_pool(name="glob", bufs=1))
        dram = ctx.enter_context(tc.tile_pool(name="dram", bufs=1, space="DRAM"))

        ag1_ins = [dram.tile([D, 32], F8, name=f"ag1i{q}") for q in range(NQ)]
        ag1_outs = [dram.tile([NCORES * D, 32], F8, addr_space="Shared",
                              name=f"ag1o{q}") for q in range(NQ)]
        ag2_in = dram.tile([2 * 128, NFRAMES], BF16)
        ag2_out = dram.tile([D, NFRAMES], BF16, addr_space="Shared")
        ag3_in = dram.tile([2 * 128, NFRAMES], BF16)
        ag3_out = dram.tile([D, NFRAMES], BF16, addr_space="Shared")

        ones_b = glob.tile([1, 512], BF16)
        nc.vector.memset(ones_b, 1.0)
        cst_q = glob.tile([1, 128], BF16)     # 1/4: step-0 e-half w scale
        nc.vector.memset(cst_q, 0.25)
        cst_16 = glob.tile([1, 128], BF16)    # 16: step-0 n-half w scale
        nc.vector.memset(cst_16, 16.0)
        cst_1k = glob.tile([1, 128], BF16)    # 1/1024: step-1 e-half w scale
        nc.vector.memset(cst_1k, 1.0 / 1024.0)
        ident = glob.tile([128, 128], BF16)
        make_identity(nc, ident)

        wl2_sb = glob.tile([128, 8], BF16)
        nc.sync.dma_start(out=wl2_sb, in_=wl2.ap())
        bl1t_sb = glob.tile([128, 8], F32)
        nc.sync.dma_start(out=bl1t_sb, in_=bl1td.ap())
        bet64_sb = glob.tile([128, 8], F32)
        nc.sync.dma_start(out=bet64_sb, in_=betd64.ap())
        bet1k_sb = glob.tile([128, 8], F32)
        nc.sync.dma_start(out=bet1k_sb, in_=betd1k.ap())

        msgn_sb = glob.tile([128, 8, FPC * O], BF16)    # [1024, 256] transposed msg_n
        msum_b = glob.tile([128, KC, FPC * H], F8)      # M_sum2^T (raw sum over o)

        with (
            tc.tile_pool(name="pwcat", bufs=1) as pwcat,
            tc.tile_pool(name="pa", bufs=2) as pa,
            tc.tile_pool(name="pa1", bufs=1) as pa1,
        ):
            # ---------------- Phase 0: msg_n^T = Wn @ O^T + bn (fp8, x64) ----------------
            wcat_sb = pwcat.tile([128, KC2, 2, D], F8)
            with (
                tc.tile_pool(name="p0", bufs=1) as p0,
                tc.tile_pool(name="p0ps", bufs=4, space="PSUM") as p0ps,
            ):
                wnb_sb = p0.tile([1, D // 2], BF16)
                nc.sync.dma_start(out=wnb_sb, in_=wnb.ap())
                ot_sb = p0.tile([128, KC2, 2, FPC * O], F8)
                nc.sync.dma_start(out=ot_sb, in_=ot.ap()
                                  .rearrange("(kc2 two p) n -> p kc2 two n", p=128, two=2))
                wn_sb = p0.tile([128, KC2, 2, D // 2], F8)
                nc.sync.dma_start(out=wn_sb, in_=wnt.ap()
                                  .rearrange("(kc2 two p) m -> p kc2 two m", p=128, two=2))
                # wcat load issued after phase-0 inputs so PE can start sooner
                nc.sync.dma_start(out=wcat_sb,
                                  in_=wcat.ap().rearrange("(kc2 two p) m -> p kc2 two m",
                                                          p=128, two=2))
                for mt in range(8):
                    pm = p0ps.tile([128, FPC * O], F32, tag="pm")
                    for kc2 in range(KC2):
                        nc.tensor.matmul(pm, lhsT=wn_sb[:, kc2, :, mt * 128:(mt + 1) * 128],
                                         rhs=ot_sb[:, kc2], start=(kc2 == 0), stop=False,
                                         perf_mode=DR)
                    nc.tensor.matmul(pm, lhsT=wnb_sb[0:1, mt * 128:(mt + 1) * 128],
                                     rhs=ones_b[0:1, 0:FPC * O], start=False, stop=True)
                    nc.scalar.activation(msgn_sb[:, mt, :], pm, AF.Copy, scale=1.0 / 64.0)

            # ---------------- Phase A: 2 propagation steps over edges ----------------
            with tc.tile_pool(name="paps", bufs=4, space="PSUM") as paps, \
                 tc.tile_pool(name="papss", bufs=2, space="PSUM") as papss:
                for q in range(NQ):
                    xq = pa.tile([128, KC2, 2, 512], F8, tag="xq")
                    nc.sync.dma_start(out=xq, in_=e0t.ap()[q]
                                      .rearrange("(kc2 two p) n -> p kc2 two n", p=128, two=2))
                    um1t = pa1.tile([128, KC2, 2, 512], F8, tag="um1t")
                    for step in range(2):
                        rhs = xq if step == 0 else um1t
                        rscale = 1.0 if step == 0 else 1.0 / 16.0
                        bet_sb = bet64_sb if step == 0 else bet1k_sb
                        cst_e = cst_q if step == 0 else cst_1k
                        cst_n = cst_16 if step == 0 else ones_b
                        # --- a-wave: relu(X @ Wl1^T + bl1), transposed ---
                        relu_sb = pa1.tile([128, 8, 512], BF16, tag="relu")
                        for mt in range(8, 16):
                            pw_a = paps.tile([128, 512], F32, tag="wave")
                            for kc2 in range(KC2):
                                nc.tensor.matmul(pw_a,
                                                 lhsT=wcat_sb[:, kc2, :, mt * 128:(mt + 1) * 128],
                                                 rhs=rhs[:, kc2], start=(kc2 == 0),
                                                 stop=(kc2 == KC2 - 1), perf_mode=DR)
                            nc.scalar.activation(relu_sb[:, mt - 8, :], pw_a, AF.Relu,
                                                 bias=bl1t_sb[:, mt - 8:mt - 7], scale=rscale)
                        # --- logits + softmax over o (groups of 16) ---
                        pl = papss.tile([1, 512], F32, tag="pl")
                        for kc2 in range(8):
                            nc.tensor.matmul(pl, lhsT=wl2_sb[:, kc2:kc2 + 1],
                                             rhs=relu_sb[:, kc2, :], start=(kc2 == 0), stop=(kc2 == 7))
                        pl3 = pl.rearrange("o (g i) -> o g i", i=16)
                        mx = pa1.tile([1, 32], F32, tag="mx")
                        nc.vector.reduce_max(mx, pl3, axis=AX.X)
                        sub = pa1.tile([1, 512], F32, tag="sub")
                        nc.vector.tensor_tensor(sub.rearrange("o (g i) -> o g i", i=16), pl3,
                                                mx.broadcast_to((1, 32, 16)), op=ALU.subtract)
                        nc.scalar.activation(sub, sub, AF.Exp)
                        ex3 = sub.rearrange("o (g i) -> o g i", i=16)
                        sm = pa1.tile([1, 32], F32, tag="sm")
                        nc.vector.reduce_sum(sm, ex3, axis=AX.X)
                        rs = pa1.tile([1, 32], F32, tag="rs")
                        nc.vector.reciprocal(rs, sm)
                        w_sb = pa1.tile([1, 512], BF16, tag="w")
                        nc.vector.tensor_tensor(w_sb.rearrange("o (g i) -> o g i", i=16), ex3,
                                                rs.broadcast_to((1, 32, 16)), op=ALU.mult)
                        # --- msg_e wave; w-broadcast MMs emitted after 2 groups ---
                        e_ps = []
                        wb_e = pa1.tile([128, 512], F32, tag="wbe")
                        wb_n = pa1.tile([128, 512], F32, tag="wbn")
                        for mt in range(8):
                            pe = paps.tile([128, 512], F32, tag="wave")
                            for kc2 in range(KC2):
                                nc.tensor.matmul(pe,
                                                 lhsT=wcat_sb[:, kc2, :, mt * 128:(mt + 1) * 128],
                                                 rhs=rhs[:, kc2], start=(kc2 == 0),
                                                 stop=(kc2 == KC2 - 1), perf_mode=DR)
                            e_ps.append(pe)
                            if mt == 1:
                                # broadcast scaled w along partitions via K=1 matmuls
                                # (PE waits here on softmax, hidden under 2 MM groups)
                                pw_b = papss.tile([128, 512], F32, tag="pw")
                                nc.tensor.matmul(pw_b, lhsT=cst_e[0:1, 0:128], rhs=w_sb,
                                                 start=True, stop=True)
                                nc.scalar.copy(wb_e, pw_b)
                                pw_c = papss.tile([128, 512], F32, tag="pw")
                                nc.tensor.matmul(pw_c, lhsT=cst_n[0:1, 0:128], rhs=w_sb,
                                                 start=True, stop=True)
                                nc.scalar.copy(wb_n, pw_c)
                            if mt >= 1:
                                for cmt in ([0, 1] if mt == 1 else [mt]):
                                    _combine_e(nc, step, cmt, q, e_ps[cmt], wb_e, bet_sb,
                                               um1t, msum_b, pa1)
                        wb4 = wb_n.rearrange("p (f h o) -> p f h o", f=4, h=8)
                        # msg_n half (tiles 8..16): broadcast over h
                        for j in range(8):
                            mt = 8 + j
                            base = msgn_sb[:, j, q * 64:(q + 1) * 64]
                            mn_bc = bass.AP(tensor=base.tensor, offset=base.offset,
                                            ap=[list(base.ap[0]), [16, 4], [0, 8], [1, 16]])
                            if step == 0:
                                nc.vector.tensor_tensor(
                                    um1t[:, mt // 2, mt % 2, :]
                                    .rearrange("p (f h o) -> p f h o", f=4, h=8),
                                    mn_bc, wb4, op=ALU.mult)
                            else:
                                tmp = pa1.tile([128, 512], F32, tag="um2")
                                nc.vector.tensor_tensor(
                                    tmp.rearrange("p (f h o) -> p f h o", f=4, h=8),
                                    mn_bc, wb4, op=ALU.mult)
                                with nc.allow_low_precision(
                                        reason="msum store bf16; o-reduce of 16 terms"):
                                    nc.vector.reduce_sum(
                                        msum_b[:, mt, q * 32:(q + 1) * 32],
                                        tmp.rearrange("p (f h o) -> p f h o", f=4, h=8),
                                        axis=AX.X)
                    # ---- AG1 chunk q: gather this quad's msum cols from all cores ----
                    nc.sync.dma_start(out=ag1_ins[q].rearrange("(kc p) n -> p kc n", p=128),
                                      in_=msum_b[:, :, q * 32:(q + 1) * 32])
                    nc.gpsimd.collective_compute(
                        "AllGather", ALU.bypass, replica_groups=RG,
                        ins=[ag1_ins[q].opt()], outs=[ag1_outs[q].opt()])

        # ---------------- Phase B: human GRU, hidden-slice parallel ----------------
        pcw = ctx.enter_context(tc.tile_pool(name="pcw", bufs=1))
        with (
            tc.tile_pool(name="pbw", bufs=1) as pbw,
            tc.tile_pool(name="pbs", bufs=2) as pbs,
            tc.tile_pool(name="pb1", bufs=1) as pb1,
            tc.tile_pool(name="pbps", bufs=2, space="PSUM") as pbps,
            tc.tile_pool(name="pbps2", bufs=2, space="PSUM") as pbps2,
        ):
            wBh_sb = pbw.tile([128, KC2, 2, 3 * HS], F8)
            nc.sync.dma_start(out=wBh_sb, in_=wBh_d.ap()
                              .rearrange("(kc2 two p) m -> p kc2 two m", p=128, two=2))
            wBi_sb = pbw.tile([128, KC2, 2, 3 * HS], F8)
            nc.sync.dma_start(out=wBi_sb, in_=wBi_d.ap()
                              .rearrange("(kc2 two p) m -> p kc2 two m", p=128, two=2))
            bB_rz = pbw.tile([1, 2 * HS], BF16)
            nc.sync.dma_start(out=bB_rz, in_=bB_rz_d.ap())
            bB_in = pbw.tile([1, HS], BF16)
            nc.sync.dma_start(out=bB_in, in_=bB_in_d.ap())
            bB_hn = pbw.tile([1, HS], BF16)
            nc.sync.dma_start(out=bB_hn, in_=bB_hn_d.ap())
            pmat_sb = pbw.tile([128, RC, NFRAMES], BF16)
            nc.sync.dma_start(out=pmat_sb, in_=pmatd.ap().rearrange("(rc p) f -> p rc f", p=128))
            hrm_sb = pbw.tile([128, RC, HS], F32)
            nc.sync.dma_start(out=hrm_sb, in_=hrms.ap().rearrange("(rc p) m -> p rc m", p=128))
            hum_sb = pbw.tile([128, RC, HS], BF16)

            # -------- phase-C gh2 (vs S_f) hoisted here: fills the AG1 stall --------
            wCh_sb = pcw.tile([128, KC, 3 * HS], BF16)
            nc.sync.dma_start(out=wCh_sb, in_=wCh_d.ap().rearrange("(kc p) m -> p kc m", p=128))
            scsf_sb = pcw.tile([128, KC, 2 * NFRAMES], BF16)
            nc.sync.dma_start(out=scsf_sb, in_=scsf_d.ap().rearrange("(kc p) n -> p kc n", p=128))
            bC_rz = pcw.tile([1, 2 * HS], BF16)
            nc.sync.dma_start(out=bC_rz, in_=bC_rz_d.ap())
            bC_hn = pcw.tile([1, HS], BF16)
            nc.sync.dma_start(out=bC_hn, in_=bC_hn_d.ap())
            g2h_rz = pcw.tile([NFRAMES, 2 * HS], BF16)
            p2h = pbps2.tile([NFRAMES, 2 * HS], F32, tag="g2a", bufs=1)
            for kc in range(KC):
                nc.tensor.matmul(p2h, lhsT=scsf_sb[:, kc, NFRAMES:2 * NFRAMES],
                                 rhs=wCh_sb[:, kc, 0:2 * HS], start=(kc == 0), stop=False)
            nc.tensor.matmul(p2h, lhsT=ones_b[0:1, 0:NFRAMES], rhs=bC_rz[0:1, :],
                             start=False, stop=True)
            nc.scalar.copy(g2h_rz, p2h)
            g2h_hn = pcw.tile([NFRAMES, HS], BF16)
            p2hn = pbps2.tile([NFRAMES, HS], F32, tag="g2b", bufs=1)
            for kc in range(KC):
                nc.tensor.matmul(p2hn, lhsT=scsf_sb[:, kc, NFRAMES:2 * NFRAMES],
                                 rhs=wCh_sb[:, kc, 2 * HS:3 * HS], start=(kc == 0), stop=False)
            nc.tensor.matmul(p2hn, lhsT=ones_b[0:1, 0:NFRAMES], rhs=bC_hn[0:1, :],
                             start=False, stop=True)
            nc.scalar.copy(g2h_hn, p2hn)

            for rc in range(RC):
                ht_t = pbs.tile([128, KC2, 2, 128], F8, tag="ht")
                nc.sync.dma_start(out=ht_t, in_=htg.ap()[:, rc * 128:(rc + 1) * 128]
                                  .rearrange("(kc2 two p) n -> p kc2 two n", p=128, two=2))
                ms_t = pbs.tile([128, KC2, 2, 128], F8, tag="ms")
                for q in range(NQ):
                    nc.sync.dma_start(out=ms_t[:, :, :, q * 32:(q + 1) * 32],
                                      in_=ag1_outs[q][rc * D:(rc + 1) * D, :]
                                      .rearrange("(kc2 two p) n -> p kc2 two n", p=128, two=2))
                p_rz = pbps.tile([128, 2 * HS], F32, tag="rz")
                for kc2 in range(KC2):
                    nc.tensor.matmul(p_rz, lhsT=ht_t[:, kc2], rhs=wBh_sb[:, kc2, :, 0:2 * HS],
                                     start=(kc2 == 0), stop=False, perf_mode=DR)
                for kc2 in range(KC2):
                    nc.tensor.matmul(p_rz, lhsT=ms_t[:, kc2], rhs=wBi_sb[:, kc2, :, 0:2 * HS],
                                     start=False, stop=False, perf_mode=DR)
                nc.tensor.matmul(p_rz, lhsT=ones_b[0:1, 0:128], rhs=bB_rz[0:1, :],
                                 start=False, stop=True)
                p_hn = pbps.tile([128, HS], F32, tag="hn", bufs=1)
                for kc2 in range(KC2):
                    nc.tensor.matmul(p_hn, lhsT=ht_t[:, kc2],
                                     rhs=wBh_sb[:, kc2, :, 2 * HS:3 * HS],
                                     start=(kc2 == 0), stop=False, perf_mode=DR)
                nc.tensor.matmul(p_hn, lhsT=ones_b[0:1, 0:128], rhs=bB_hn[0:1, :],
                                 start=False, stop=True)
                p_in = pbps.tile([128, HS], F32, tag="in", bufs=1)
                for kc2 in range(KC2):
                    nc.tensor.matmul(p_in, lhsT=ms_t[:, kc2],
                                     rhs=wBi_sb[:, kc2, :, 2 * HS:3 * HS],
                                     start=(kc2 == 0), stop=False, perf_mode=DR)
                nc.tensor.matmul(p_in, lhsT=ones_b[0:1, 0:128], rhs=bB_in[0:1, :],
                                 start=False, stop=True)
                r_sb = pb1.tile([128, HS], F32, tag="r")
                nc.scalar.activation(r_sb, p_rz[:, 0:HS], AF.Sigmoid, scale=1.0 / 64.0)
                z_sb = pb1.tile([128, HS], F32, tag="z")
                nc.scalar.activation(z_sb, p_rz[:, HS:2 * HS], AF.Sigmoid, scale=1.0 / 64.0)
                t1 = pb1.tile([128, HS], F32, tag="t1")
                nc.vector.tensor_tensor(t1, r_sb, p_hn, op=ALU.mult)
                t2 = pb1.tile([128, HS], F32, tag="t2")
                nc.vector.tensor_tensor(t2, t1, p_in, op=ALU.add)
                n_sb = pb1.tile([128, HS], F32, tag="n")
                nc.scalar.activation(n_sb, t2, AF.Tanh, scale=1.0 / 64.0)
                t3 = pb1.tile([128, HS], F32, tag="t3")
                nc.vector.tensor_tensor(t3, hrm_sb[:, rc, :], n_sb, op=ALU.subtract)
                t4 = pb1.tile([128, HS], F32, tag="t4")
                nc.vector.tensor_tensor(t4, z_sb, t3, op=ALU.mult)
                nc.vector.tensor_tensor(hum_sb[:, rc, :], n_sb, t4, op=ALU.add)

            # All_human^T slice: ah[mc] = hum[:, mc-chunk].T @ pmat   [256, 128]
            ahT_sb = pb1.tile([128, 2, NFRAMES], BF16, tag="ahT")
            for mc in range(2):
                pah = pbps2.tile([128, NFRAMES], F32, tag="pah", bufs=1)
                for rc in range(RC):
                    nc.tensor.matmul(pah, lhsT=hum_sb[:, rc, mc * 128:(mc + 1) * 128],
                                     rhs=pmat_sb[:, rc, :], start=(rc == 0), stop=(rc == RC - 1))
                nc.scalar.copy(ahT_sb[:, mc, :], pah)
            nc.sync.dma_start(out=ag2_in.rearrange("(mc p) f -> p mc f", p=128),
                              in_=ahT_sb)
            nc.gpsimd.collective_compute(
                "AllGather", ALU.bypass, replica_groups=RG,
                ins=[ag2_in.opt()], outs=[ag2_out.opt()])

            # -------- remaining Phase C prefetches (loaded during B) --------
            wCi_sb = pcw.tile([128, KC, 3 * HS], BF16)
            nc.sync.dma_start(out=wCi_sb, in_=wCi_d.ap().rearrange("(kc p) m -> p kc m", p=128))
            bC_in = pcw.tile([1, HS], BF16)
            nc.sync.dma_start(out=bC_in, in_=bC_in_d.ap())
            sc4s_sb = pcw.tile([NFRAMES, HS], F32)
            nc.sync.dma_start(out=sc4s_sb, in_=sc4s_d.ap())
            sfs_sb = pcw.tile([NFRAMES, HS], F32)
            nc.sync.dma_start(out=sfs_sb, in_=sfs_d.ap())

        # ---------------- Phase C: two S-node GRUs, hidden-slice parallel ----------------
        with (
            tc.tile_pool(name="pc1", bufs=1) as pc1,
            tc.tile_pool(name="pcps", bufs=2, space="PSUM") as pcps,
            tc.tile_pool(name="pctps", bufs=2, space="PSUM") as pctps,
        ):
            ah_sb = pc1.tile([128, KC, NFRAMES], BF16)
            nc.sync.dma_start(out=ah_sb, in_=ag2_out
                              .rearrange("(kc p) f -> p kc f", p=128))

            # step-1 gates: gh1(S_C4) first (AG2-independent), then gi1(AH)
            p1_hn = pcps.tile([NFRAMES, HS], F32, tag="hn")
            for kc in range(KC):
                nc.tensor.matmul(p1_hn, lhsT=scsf_sb[:, kc, 0:NFRAMES],
                                 rhs=wCh_sb[:, kc, 2 * HS:3 * HS], start=(kc == 0), stop=False)
            nc.tensor.matmul(p1_hn, lhsT=ones_b[0:1, 0:NFRAMES], rhs=bC_hn[0:1, :],
                             start=False, stop=True)
            p1_rz = pcps.tile([NFRAMES, 2 * HS], F32, tag="rz")
            for kc in range(KC):
                nc.tensor.matmul(p1_rz, lhsT=scsf_sb[:, kc, 0:NFRAMES],
                                 rhs=wCh_sb[:, kc, 0:2 * HS], start=(kc == 0), stop=False)
            for kc in range(KC):
                nc.tensor.matmul(p1_rz, lhsT=ah_sb[:, kc, :], rhs=wCi_sb[:, kc, 0:2 * HS],
                                 start=False, stop=False)
            nc.tensor.matmul(p1_rz, lhsT=ones_b[0:1, 0:NFRAMES], rhs=bC_rz[0:1, :],
                             start=False, stop=True)
            p1_in = pcps.tile([NFRAMES, HS], F32, tag="in")
            for kc in range(KC):
                nc.tensor.matmul(p1_in, lhsT=ah_sb[:, kc, :], rhs=wCi_sb[:, kc, 2 * HS:3 * HS],
                                 start=(kc == 0), stop=False)
            nc.tensor.matmul(p1_in, lhsT=ones_b[0:1, 0:NFRAMES], rhs=bC_in[0:1, :],
                             start=False, stop=True)

            # step-1 elementwise -> s1 slice
            z1 = pc1.tile([NFRAMES, HS], F32, tag="z1")
            nc.scalar.activation(z1, p1_rz[:, HS:2 * HS], AF.Sigmoid)
            r1 = pc1.tile([NFRAMES, HS], F32, tag="r1")
            nc.scalar.activation(r1, p1_rz[:, 0:HS], AF.Sigmoid)
            u1 = pc1.tile([NFRAMES, HS], F32, tag="u1")
            nc.vector.tensor_tensor(u1, r1, p1_hn, op=ALU.mult)
            u2 = pc1.tile([NFRAMES, HS], F32, tag="u2")
            nc.vector.tensor_tensor(u2, u1, p1_in, op=ALU.add)
            n1 = pc1.tile([NFRAMES, HS], F32, tag="n1")
            nc.scalar.activation(n1, u2, AF.Tanh)
            u3 = pc1.tile([NFRAMES, HS], F32, tag="u3")
            nc.vector.tensor_tensor(u3, sc4s_sb, n1, op=ALU.subtract)
            u4 = pc1.tile([NFRAMES, HS], F32, tag="u4")
            nc.vector.tensor_tensor(u4, z1, u3, op=ALU.mult)
            s1_sb = pc1.tile([NFRAMES, HS], BF16, tag="s1")
            nc.vector.tensor_tensor(s1_sb, n1, u4, op=ALU.add)

            # transpose s1 slice -> [256, 128] and AllGather full s1^T
            s1t_sb = pc1.tile([128, 2, NFRAMES], BF16, tag="s1t")
            for mc in range(2):
                ptp = pctps.tile([128, NFRAMES], BF16, tag="tp")
                nc.tensor.transpose(ptp, s1_sb[:, mc * 128:(mc + 1) * 128], ident)
                nc.scalar.copy(s1t_sb[:, mc, :], ptp)
            nc.sync.dma_start(out=ag3_in.rearrange("(mc p) f -> p mc f", p=128),
                              in_=s1t_sb)
            nc.gpsimd.collective_compute(
                "AllGather", ALU.bypass, replica_groups=RG,
                ins=[ag3_in.opt()], outs=[ag3_out.opt()])
            s1t_g = pc1.tile([128, KC, NFRAMES], BF16)
            nc.sync.dma_start(out=s1t_g, in_=ag3_out
                              .rearrange("(kc p) f -> p kc f", p=128))

            # step-2 gates: gi2(s1); gh2 already in SBUF
            p2_rz = pcps.tile([NFRAMES, 2 * HS], F32, tag="rz")
            for kc in range(KC):
                nc.tensor.matmul(p2_rz, lhsT=s1t_g[:, kc, :], rhs=wCi_sb[:, kc, 0:2 * HS],
                                 start=(kc == 0), stop=(kc == KC - 1))
            p2_n = pcps.tile([NFRAMES, HS], F32, tag="in")
            for kc in range(KC):
                nc.tensor.matmul(p2_n, lhsT=s1t_g[:, kc, :], rhs=wCi_sb[:, kc, 2 * HS:3 * HS],
                                 start=(kc == 0), stop=False)
            nc.tensor.matmul(p2_n, lhsT=ones_b[0:1, 0:NFRAMES], rhs=bC_in[0:1, :],
                             start=False, stop=True)

            # step-2 elementwise -> out slice
            grz = pc1.tile([NFRAMES, 2 * HS], F32, tag="grz")
            nc.vector.tensor_tensor(grz, p2_rz, g2h_rz, op=ALU.add)
            z2 = pc1.tile([NFRAMES, HS], F32, tag="z2")
            nc.scalar.activation(z2, grz[:, HS:2 * HS], AF.Sigmoid)
            r2 = pc1.tile([NFRAMES, HS], F32, tag="r2")
            nc.scalar.activation(r2, grz[:, 0:HS], AF.Sigmoid)
            v1 = pc1.tile([NFRAMES, HS], F32, tag="v1")
            nc.vector.tensor_tensor(v1, r2, g2h_hn, op=ALU.mult)
            v2 = pc1.tile([NFRAMES, HS], F32, tag="v2")
            nc.vector.tensor_tensor(v2, v1, p2_n, op=ALU.add)
            n2 = pc1.tile([NFRAMES, HS], F32, tag="n2")
            nc.scalar.activation(n2, v2, AF.Tanh)
            v3 = pc1.tile([NFRAMES, HS], F32, tag="v3")
            nc.vector.tensor_tensor(v3, sfs_sb, n2, op=ALU.subtract)
            v4 = pc1.tile([NFRAMES, HS], F32, tag="v4")
            nc.vector.tensor_tensor(v4, z2, v3, op=ALU.mult)
            out_sb = pc1.tile([NFRAMES, HS], F32, tag="out")
            nc.vector.tensor_tensor(out_sb, n2, v4, op=ALU.add)
            nc.sync.dma_start(out=outp.ap(), in_=out_sb)

    nc.compile()
    return nc


def _prep_in_maps(inputs):
    E = np.ascontiguousarray(inputs["H_O_edges"].reshape(NFRAMES, ROWS, D))
    On = inputs["O_nodes"].reshape(NFRAMES, O, D)
    Hn = inputs["H_nodes"].reshape(NFRAMES, H, D)
    Sc4 = inputs["S_node_C4"].reshape(NFRAMES, D)
    Sf = np.ascontiguousarray(inputs["final_S_node"].transpose(0, 2, 1)).reshape(NFRAMES, D)
    Hn_rm = Hn.reshape(NR, D)  # rows = (frame, h)

    bB_rz_full = inputs["gh_bih"] + inputs["gh_bhh"]
    bC_rz_full = inputs["gs_bih"] + inputs["gs_bhh"]

    shared = {
        "wcat": np.ascontiguousarray(
            64.0 * np.concatenate([inputs["We"], inputs["Wl1"]], axis=0).T).astype(NF8),
        "bl1t": np.ascontiguousarray(64.0 * inputs["bl1"].reshape(8, 128).T).astype(np.float32),
        "bet64": np.ascontiguousarray(64.0 * inputs["be"].reshape(8, 128).T).astype(np.float32),
        "bet1k": np.ascontiguousarray(1024.0 * inputs["be"].reshape(8, 128).T).astype(np.float32),
        "wnt": np.ascontiguousarray(64.0 * inputs["Wn"].T).astype(NF8),
        "wnb": (64.0 * inputs["bn"])[None, :].astype(NB),
        "wl2": np.ascontiguousarray(inputs["Wl2"][0].reshape(8, 128).T / 64.0).astype(NB),
        "htg": np.ascontiguousarray(Hn_rm.T).astype(NF8),
        "pmat": np.ascontiguousarray(np.kron(np.eye(NFRAMES), np.ones((H, 1))) / H).astype(NB),
        "scsf": np.ascontiguousarray(np.concatenate([Sc4.T, Sf.T], axis=1)).astype(NB),
    }

    in_maps = []
    for c in range(NCORES):
        fr = slice(c * FPC, (c + 1) * FPC)
        Ec = E[fr]  # [16, 128, 2048]
        e0t = np.ascontiguousarray(
            Ec.reshape(NQ, 4, ROWS, D).transpose(0, 3, 1, 2).reshape(NQ, D, 512)).astype(NF8)
        ot = np.ascontiguousarray(
            On[fr].reshape(FPC * O, D).T).astype(NF8)
        hs = slice(c * HS, (c + 1) * HS)
        rows_rzn = np.r_[c * HS:(c + 1) * HS,
                         D + c * HS:D + (c + 1) * HS,
                         2 * D + c * HS:2 * D + (c + 1) * HS]
        rows_rz = rows_rzn[:2 * HS]
        m = dict(shared)
        m.update({
            "e0t": e0t,
            "ot": ot,
            "hrms": np.ascontiguousarray(Hn_rm[:, hs]).astype(np.float32),
            "wBi": np.ascontiguousarray(
                (64.0 / O) * inputs["gh_wih"][rows_rzn].T).astype(NF8),
            "wBh": np.ascontiguousarray(64.0 * inputs["gh_whh"][rows_rzn].T).astype(NF8),
            "bB_rz": (64.0 * bB_rz_full[rows_rz])[None, :].astype(NB),
            "bB_in": (64.0 * inputs["gh_bih"][rows_rzn[2 * HS:]])[None, :].astype(NB),
            "bB_hn": (64.0 * inputs["gh_bhh"][rows_rzn[2 * HS:]])[None, :].astype(NB),
            "sc4s": np.ascontiguousarray(Sc4[:, hs]).astype(np.float32),
            "sfs": np.ascontiguousarray(Sf[:, hs]).astype(np.float32),
            "wCi": np.ascontiguousarray(inputs["gs_wih"][rows_rzn].T).astype(NB),
            "wCh": np.ascontiguousarray(inputs["gs_whh"][rows_rzn].T).astype(NB),
            "bC_rz": bC_rz_full[rows_rz][None, :].astype(NB),
            "bC_in": inputs["gs_bih"][rows_rzn[2 * HS:]][None, :].astype(NB),
            "bC_hn": inputs["gs_bhh"][rows_rzn[2 * HS:]][None, :].astype(NB),
        })
        in_maps.append(m)
    return in_maps


LAST_RESULT = None


def kernel(**inputs):
    global LAST_RESULT
    if "nc" not in _CACHE:
        _CACHE["nc"] = _build_nc()
    nc = _CACHE["nc"]
    in_maps = _prep_in_maps(inputs)
    trace = os.environ.get("KERNEL_TRACE", "0") == "1"
    res = bass_utils.run_bass_kernel_spmd(
        nc, in_maps, core_ids=list(range(NCORES)), trace=trace)
    LAST_RESULT = res
    out = np.empty((NFRAMES, D), np.float32)
    for c in range(NCORES):
        out[:, c * HS:(c + 1) * HS] = res.results[c]["outp"]
    return np.ascontiguousarray(out.reshape(B, F, D)).astype(np.float32)


if __name__ == "__main__":
    np.random.seed(0)
    ins = {
        "S_node_C4": np.random.randn(B, F, D).astype(np.float32),
        "final_S_node": np.random.randn(B, D, F).astype(np.float32),
        "H_nodes": np.random.randn(B, F, H, D).astype(np.float32),
        "O_nodes": np.random.randn(B, F, O, D).astype(np.float32),
        "H_O_edges": np.random.randn(B, F, H, O, D).astype(np.float32),
        "Wn": np.random.randn(D // 2, D).astype(np.float32) * 0.02,
        "bn": np.random.randn(D // 2).astype(np.float32) * 0.02,
        "We": np.random.randn(D // 2, D).astype(np.float32) * 0.02,
        "be": np.random.randn(D // 2).astype(np.float32) * 0.02,
        "Wl1": np.random.randn(D // 2, D).astype(np.float32) * 0.02,
        "bl1": np.random.randn(D // 2).astype(np.float32) * 0.02,
        "Wl2": np.random.randn(1, D // 2).astype(np.float32) * 0.02,
        "bl2": np.random.randn(1).astype(np.float32) * 0.02,
        "gh_wih": np.random.randn(3 * D, D).astype(np.float32) * 0.02,
        "gh_whh": np.random.randn(3 * D, D).astype(np.float32) * 0.02,
        "gh_bih": np.random.randn(3 * D).astype(np.float32) * 0.02,
        "gh_bhh": np.random.randn(3 * D).astype(np.float32) * 0.02,
        "gs_wih": np.random.randn(3 * D, D).astype(np.float32) * 0.02,
        "gs_whh": np.random.randn(3 * D, D).astype(np.float32) * 0.02,
        "gs_bih": np.random.randn(3 * D).astype(np.float32) * 0.02,
        "gs_bhh": np.random.randn(3 * D).astype(np.float32) * 0.02,
    }
    out = kernel(**ins)
    print("kernel ran, out shape", out.shape, out.dtype, float(np.abs(out).mean()))
